# revision 1
# baseline (speedup 1.0000x reference)
"""Trainium2 Bass kernel for nn_ReasonerModel (12-layer cross-attn transformer).

Sharding: pure data-parallel over batch. 32 batch elems / 8 cores = 4 per core.
Each core holds the full weights (streamed from its HBM) and computes its 4
batch rows end-to-end; no collectives. Matmuls run in bf16 with fp32 PSUM
accumulation; the residual stream / layernorms stay fp32.

Layout conventions per core (B_loc = 4, SQ = 80, SKV = 1024, D = 1024, H = 16):
  x_b      [80, 1024] f32, per b    natural residual stream (LN-friendly)
  hT/pT    [128, 8, 4, 80] bf16     transposed activations (d on partitions)
  knowT    [4, 8, 128, 1024] bf16   pre-transposed know (DRAM, built in prologue)
  kT_b     [128, 8, 1024] bf16      per-b K^T   (n on partitions, s free)
  v_b      [128, 8, 1024] bf16      per-b V     (s on partitions, n free)
  aT       [128, 8, 4, 80] bf16     attention out, transposed
  gT       [128, 32, 4, 80] bf16    2*gelu(fc) transposed (0.5 folded into Wm)
All projections compute out^T = W^T-tiles @ xT so biases land on partitions.
"""

import os
import sys

sys.path.insert(0, "/opt/trn_rl_repo")

import numpy as np

import concourse.bass as bass
import concourse.tile as tile
from concourse import mybir
from concourse.bass_utils import run_bass_kernel_spmd
from concourse.masks import make_identity
from concourse.vector_clock import ScopedClock

# model dims (fixed by the problem)
B, SQ, SKV, D, H = 32, 80, 1024, 1024, 16
L = int(os.environ.get("KERNEL_LAYERS", "12"))
REPEAT = int(os.environ.get("KERNEL_REPEAT", "1"))  # timing calibration only
HD = D // H          # 64
N_CORES = 8
BL = B // N_CORES    # 4 batch rows per core
DT = D // 128        # 8 d-tiles
FT = 4 * D // 128    # 32 ffn tiles
EPS = 1e-5
GELU_C = 0.044715
GELU_S = 0.7978845608028654  # sqrt(2/pi)
GELU_LUT = os.environ.get("GELU_LUT", "0") == "1"
KV_FP8 = os.environ.get("KV_FP8", "0") == "1"
FP8_SCALE = 64.0            # pre-scale know/Wk/Wv into e4m3 range
FP8_INV = 1.0 / (FP8_SCALE * FP8_SCALE)

F32 = mybir.dt.float32
BF16 = mybir.dt.bfloat16
FP8 = mybir.dt.float8e4
KVDT = FP8 if KV_FP8 else BF16
AF = mybir.ActivationFunctionType
ALU = mybir.AluOpType
AX = mybir.AxisListType


class PatchedTC(tile.TileContext):
    """This container's walrus accepts at most ONE sem wait per instruction;
    Tile may attach several. Peel extras onto preceding same-engine no-ops."""

    def _commit_instruction(self, inst, lazy_reg_writes: bool = True):
        si = getattr(inst, "sync_info", None)
        if (
            si is not None
            and si.on_wait
            and len(si.on_wait) > 1
            and inst.engine != mybir.EngineType.Unassigned
        ):
            waits = list(si.on_wait)
            si.on_wait = [waits[-1]]
            for j, w in enumerate(waits[:-1]):
                nop = mybir.InstNoOp(
                    name=f"{inst.name}-sw{j}",
                    sync_info=mybir.SyncInfo(on_wait=[w], on_update=[]),
                    bass_nofuse=True,
                    engine=inst.engine,
                )
                super()._commit_instruction(nop, lazy_reg_writes=False)
        return super()._commit_instruction(inst, lazy_reg_writes)

    def _drain_and_barrier(self, tick_clock, wait_clock):
        drain_inst = self.nc.sync.drain()
        wait_clock.add_sem_waits(
            drain_inst.ins, ScopedClock({None: tick_clock.global_clock})
        )
        si = drain_inst.ins.sync_info
        if si is not None and si.on_wait and len(si.on_wait) > 1:
            waits = list(si.on_wait)
            si.on_wait = waits[:1]
            for w in waits[1:]:
                extra = self.nc.sync.drain()
                nsi = extra.ins.sync_info
                if nsi is None:
                    extra.ins.sync_info = mybir.SyncInfo(on_wait=[w], on_update=[])
                else:
                    nsi.on_wait = [w]
        self.nc.all_engine_barrier()
        assert self.sems is not None
        popped = self.nc._tile_sem_poison_stack.pop()
        assert popped is self._sem_poison
        self.nc.clear_and_free_semaphores(list(self.sems.allocated().values()))
        self.nc.all_engine_barrier()


def bcast_ap(ap_1d, p):
    """Partition-broadcast a 1-D DRAM AP to [p, n] (stride-0 partition dim)."""
    return bass.AP(
        tensor=ap_1d.tensor, offset=ap_1d.offset, ap=[[0, p]] + list(ap_1d.ap)
    )


def build_nc():
    try:  # lift the stale 192KB/partition SBUF cap to the real usable 208KB
        from concourse import tile_utils

        tile_utils.max_sbuf_usage = 208 * 1024
    except Exception:
        pass

    nc = bass.Bass("TRN2", target_bir_lowering=False, debug=False,
                   num_devices=N_CORES)

    # ---- DRAM I/O (per-core shard for acts, replicated weights) ----
    x_in = nc.dram_tensor("input_ids", [BL, SQ, D], F32, kind="ExternalInput")
    know_in = nc.dram_tensor("input_ids_know", [BL, SKV, D], F32,
                             kind="ExternalInput")
    pos_in = nc.dram_tensor("pos_embed", [SQ, D], F32, kind="ExternalInput")
    Wa = nc.dram_tensor("W_attn", [L, D, 3 * D], F32, kind="ExternalInput")
    ba = nc.dram_tensor("b_attn", [L, 3 * D], F32, kind="ExternalInput")
    Wp = nc.dram_tensor("W_proj_attn", [L, D, D], F32, kind="ExternalInput")
    bp = nc.dram_tensor("b_proj_attn", [L, D], F32, kind="ExternalInput")
    g1 = nc.dram_tensor("ln1_g", [L, D], F32, kind="ExternalInput")
    b1 = nc.dram_tensor("ln1_b", [L, D], F32, kind="ExternalInput")
    Wf = nc.dram_tensor("W_fc", [L, D, 4 * D], F32, kind="ExternalInput")
    bf = nc.dram_tensor("b_fc", [L, 4 * D], F32, kind="ExternalInput")
    Wm = nc.dram_tensor("W_proj_mlp", [L, 4 * D, D], F32, kind="ExternalInput")
    bm = nc.dram_tensor("b_proj_mlp", [L, D], F32, kind="ExternalInput")
    g2 = nc.dram_tensor("ln2_g", [L, D], F32, kind="ExternalInput")
    b2 = nc.dram_tensor("ln2_b", [L, D], F32, kind="ExternalInput")
    out_ext = nc.dram_tensor("out", [BL, SQ, D], F32, kind="ExternalOutput")

    knowT_dram = nc.dram_tensor("knowT", [BL, DT, 128, SKV], KVDT)

    with PatchedTC(nc) as tc:
        import contextlib

        ctx = contextlib.ExitStack()
        with ctx:
            P = lambda **kw: ctx.enter_context(tc.tile_pool(**kw))
            singles = P(name="singles", bufs=1)
            xT_pool = P(name="xT", bufs=2)
            aT_pool = P(name="aT", bufs=1)
            oT_pool = P(name="oT", bufs=1)          # aoutT / moutT
            gT_pool = P(name="gT", bufs=1)
            kv_pool = P(name="kv", bufs=1)
            knb_pool = P(name="knb", bufs=1)
            wkv_pool = P(name="wkv", bufs=1)
            wch_pool = P(name="wch", bufs=2)        # wp/wf/wm chunks by tag
            stg_pool = P(name="stg", bufs=3)
            w_pool = P(name="wsm", bufs=3)          # softmax weights
            wT_pool = P(name="wT", bufs=4)
            tt_pool = P(name="tt", bufs=3)          # [128,128] transpose bounce
            gel_pool = P(name="gel", bufs=2)
            st_pool = P(name="st", bufs=6)          # tiny stats tiles
            bc_pool = P(name="bc", bufs=1)          # per-layer bcast vectors
            sb_pool = P(name="sb", bufs=2)          # per-layer small biases
            psA = P(name="psA", bufs=4, space="PSUM")
            psB = P(name="psB", bufs=2, space="PSUM")

            # ---- constants ----
            id_bf = singles.tile([128, 128], BF16)
            make_identity(nc, id_bf)
            id_f32 = singles.tile([128, 128], F32)
            make_identity(nc, id_f32)
            eps_t = singles.tile([128, 1], F32)
            nc.vector.memset(eps_t, EPS)

            # ---- residual-stream tiles (persistent) ----
            xs = [
                singles.tile([SQ, D], F32, tag=f"x{b}", name=f"x{b}")
                for b in range(BL)
            ]

            def ln(x_b, g_bc, b_bc):
                stt = st_pool.tile([SQ, 2, 6], F32, tag="bnst")
                mv = st_pool.tile([SQ, 2], F32, tag="bnmv")
                for c in range(2):
                    nc.vector.bn_stats(stt[:, c, :], x_b[:, c * 512:(c + 1) * 512])
                nc.vector.bn_aggr(mv, stt)
                std = st_pool.tile([SQ, 1], F32, tag="bnsd")
                nc.scalar.activation(std, mv[:, 1:2], AF.Sqrt, bias=eps_t[:SQ])
                nc.vector.reciprocal(std, std)
                nc.vector.tensor_scalar(x_b, x_b, mv[:, 0:1], std,
                                        op0=ALU.subtract, op1=ALU.mult)
                nc.gpsimd.tensor_tensor(x_b, x_b, g_bc[:SQ, :], ALU.mult)
                nc.gpsimd.tensor_tensor(x_b, x_b, b_bc[:SQ, :], ALU.add)

            def transpose_nat_to_T(x_b, dstT, b, cast_pool):
                """x_b [80, 1024] f32 -> dstT[:, dt, b, :] bf16 (PE transpose)."""
                for dt in range(DT):
                    pt = psA.tile([128, 512], F32, tag="psA")
                    nc.tensor.transpose(pt[:, :SQ], x_b[:, dt * 128:(dt + 1) * 128],
                                        id_f32[:SQ, :SQ])
                    eng = nc.vector if dt % 2 == 0 else nc.scalar
                    if eng is nc.vector:
                        nc.vector.tensor_copy(out=dstT[:, dt, b, :], in_=pt[:, :SQ])
                    else:
                        nc.scalar.copy(out=dstT[:, dt, b, :], in_=pt[:, :SQ])

            # ================= prologue =================
            pos_sb = singles.tile([SQ, D], F32, tag="pos")
            nc.sync.dma_start(out=pos_sb, in_=pos_in[:, :])
            # pre-transpose know -> knowT_dram (bf16)
            for b in range(BL):
                for stt in range(DT):
                    stg = stg_pool.tile([128, D], F32, tag="stg")
                    nc.sync.dma_start(
                        out=stg, in_=know_in[b, stt * 128:(stt + 1) * 128, :])
                    ktmp = w_pool.tile([128, D], BF16, tag="w")
                    if KV_FP8:
                        nc.scalar.activation(out=ktmp, in_=stg, func=AF.Copy,
                                             scale=FP8_SCALE)
                    else:
                        nc.vector.tensor_copy(out=ktmp, in_=stg)
                    for dt in range(DT):
                        pt = psA.tile([128, 512], BF16, tag="psA")
                        nc.tensor.transpose(
                            pt[:, :128], ktmp[:, dt * 128:(dt + 1) * 128], id_bf)
                        kout = tt_pool.tile([128, 128], KVDT, tag="tt")
                        if dt % 2 == 0:
                            nc.vector.tensor_copy(out=kout, in_=pt[:, :128])
                        else:
                            nc.scalar.copy(out=kout, in_=pt[:, :128])
                        nc.sync.dma_start(
                            out=knowT_dram[b, dt, :, stt * 128:(stt + 1) * 128],
                            in_=kout)

            for rep in range(REPEAT):
                hT = xT_pool.tile([128, DT, BL, SQ], BF16, tag="xT")
                for b in range(BL):
                    nc.sync.dma_start(out=xs[b], in_=x_in[b])
                    nc.vector.tensor_add(xs[b], xs[b], pos_sb)
                    transpose_nat_to_T(xs[b], hT, b, tt_pool)

                # ================= layers =================
                for l in range(L):
                    # ---- per-layer broadcast / bias tiles ----
                    def bvec(src_ap, tag):  # [D] f32 -> [128, D] bf16 broadcast
                        stg = stg_pool.tile([128, D], F32, tag="stg")
                        nc.gpsimd.dma_start(out=stg, in_=bcast_ap(src_ap, 128))
                        t = bc_pool.tile([128, D], BF16, tag=tag)
                        nc.gpsimd.tensor_copy(out=t, in_=stg)
                        return t

                    bv_bc = bvec(ba[l, 2 * D:3 * D], "bv")
                    g1_bc = bvec(g1[l], "g1")
                    b1_bc = bvec(b1[l], "b1")
                    g2_bc = bvec(g2[l], "g2")
                    b2_bc = bvec(b2[l], "b2")
                    bk_sb = sb_pool.tile([128, DT], F32, tag="bk")
                    nc.sync.dma_start(
                        out=bk_sb, in_=ba[l, D:2 * D].rearrange("(t p) -> p t", p=128))
                    bp_sb = sb_pool.tile([128, DT], F32, tag="bp")
                    nc.sync.dma_start(
                        out=bp_sb, in_=bp[l].rearrange("(t p) -> p t", p=128))
                    bm_sb = sb_pool.tile([128, DT], F32, tag="bm")
                    nc.sync.dma_start(
                        out=bm_sb, in_=bm[l].rearrange("(t p) -> p t", p=128))
                    bf_sb = sb_pool.tile([128, FT], F32, tag="bf")
                    nc.sync.dma_start(
                        out=bf_sb, in_=bf[l].rearrange("(t p) -> p t", p=128))

                    # ---- stream Wk/Wv (bf16, full per layer) ----
                    wk_sb = wkv_pool.tile([128, DT, D], KVDT, tag="wk")
                    wv_sb = wkv_pool.tile([128, DT, D], KVDT, tag="wv")
                    Wa_l = Wa[l].rearrange("(t p) n -> p t n", p=128)  # [128,8,3D]
                    for c in range(DT):
                        stg = stg_pool.tile([128, DT, 128], F32, tag="stg")
                        nc.sync.dma_start(
                            out=stg, in_=Wa_l[:, :, D + c * 128:D + (c + 1) * 128])
                        if KV_FP8:
                            nc.scalar.activation(
                                out=wk_sb[:, :, c * 128:(c + 1) * 128],
                                in_=stg, func=AF.Copy, scale=FP8_SCALE)
                        elif c % 2 == 0:
                            nc.vector.tensor_copy(
                                out=wk_sb[:, :, c * 128:(c + 1) * 128], in_=stg)
                        else:
                            nc.gpsimd.tensor_copy(
                                out=wk_sb[:, :, c * 128:(c + 1) * 128], in_=stg)
                    for c in range(DT):
                        stg = stg_pool.tile([128, DT, 128], F32, tag="stg")
                        nc.sync.dma_start(
                            out=stg,
                            in_=Wa_l[:, :, 2 * D + c * 128:2 * D + (c + 1) * 128])
                        if KV_FP8:
                            nc.vector.tensor_single_scalar(
                                out=wv_sb[:, :, c * 128:(c + 1) * 128],
                                in_=stg, scalar=FP8_SCALE, op=ALU.mult)
                        elif c % 2 == 0:
                            nc.gpsimd.tensor_copy(
                                out=wv_sb[:, :, c * 128:(c + 1) * 128], in_=stg)
                        else:
                            nc.vector.tensor_copy(
                                out=wv_sb[:, :, c * 128:(c + 1) * 128], in_=stg)

                    aT = aT_pool.tile([128, DT, BL, SQ], BF16, tag="aT")

                    # ---- per-batch kv + attention ----
                    for b in range(BL):
                        knb = knb_pool.tile([128, DT, SKV], KVDT, tag="knb")
                        nc.sync.dma_start(
                            out=knb, in_=knowT_dram[b].rearrange("t p s -> p t s"))

                        # K^T: [n-part, s]
                        kTb = kv_pool.tile([128, DT, SKV], BF16, tag="kT")
                        for nt in range(DT):
                            for sc in range(2):
                                ps = psA.tile([128, 512], F32, tag="psA")
                                if KV_FP8:
                                    for k2 in range(DT // 2):
                                        nc.tensor.matmul(
                                            ps,
                                            lhsT=wk_sb[:, 2 * k2:2 * k2 + 2,
                                                       nt * 128:(nt + 1) * 128],
                                            rhs=knb[:, 2 * k2:2 * k2 + 2,
                                                    sc * 512:(sc + 1) * 512],
                                            start=(k2 == 0),
                                            stop=(k2 == DT // 2 - 1),
                                            perf_mode=mybir.MatmulPerfMode.DoubleRow)
                                else:
                                    for kt in range(DT):
                                        nc.tensor.matmul(
                                            ps,
                                            lhsT=wk_sb[:, kt,
                                                       nt * 128:(nt + 1) * 128],
                                            rhs=knb[:, kt, sc * 512:(sc + 1) * 512],
                                            start=(kt == 0), stop=(kt == DT - 1))
                                if sc == 0:
                                    nc.scalar.activation(
                                        out=kTb[:, nt, sc * 512:(sc + 1) * 512],
                                        in_=ps, func=AF.Identity,
                                        scale=FP8_INV if KV_FP8 else 1.0,
                                        bias=bk_sb[:, nt:nt + 1])
                                else:
                                    nc.vector.tensor_scalar(
                                        out=kTb[:, nt, sc * 512:(sc + 1) * 512],
                                        in0=ps,
                                        scalar1=FP8_INV if KV_FP8 else 1.0,
                                        scalar2=bk_sb[:, nt:nt + 1],
                                        op0=ALU.mult, op1=ALU.add)

                        # V: [s-part, n]
                        vb = kv_pool.tile([128, DT, D], BF16, tag="v")
                        for stv in range(DT):
                            for nc2 in range(2):
                                ps = psA.tile([128, 512], F32, tag="psA")
                                if KV_FP8:
                                    for k2 in range(DT // 2):
                                        nc.tensor.matmul(
                                            ps,
                                            lhsT=knb[:, 2 * k2:2 * k2 + 2,
                                                     stv * 128:(stv + 1) * 128],
                                            rhs=wv_sb[:, 2 * k2:2 * k2 + 2,
                                                      nc2 * 512:(nc2 + 1) * 512],
                                            start=(k2 == 0),
                                            stop=(k2 == DT // 2 - 1),
                                            perf_mode=mybir.MatmulPerfMode.DoubleRow)
                                    nc.vector.scalar_tensor_tensor(
                                        out=vb[:, stv, nc2 * 512:(nc2 + 1) * 512],
                                        in0=ps, scalar=FP8_INV,
                                        in1=bv_bc[:, nc2 * 512:(nc2 + 1) * 512],
                                        op0=ALU.mult, op1=ALU.add)
                                else:
                                    for kt in range(DT):
                                        nc.tensor.matmul(
                                            ps,
                                            lhsT=knb[:, kt,
                                                     stv * 128:(stv + 1) * 128],
                                            rhs=wv_sb[:, kt,
                                                      nc2 * 512:(nc2 + 1) * 512],
                                            start=(kt == 0), stop=(kt == DT - 1))
                                    nc.vector.tensor_tensor(
                                        vb[:, stv, nc2 * 512:(nc2 + 1) * 512], ps,
                                        bv_bc[:, nc2 * 512:(nc2 + 1) * 512],
                                        ALU.add)

                        # attention, head-pair at a time
                        for hp in range(DT):
                            wTs = []
                            for hs in range(2):
                                po = hs * 64
                                scp = psB.tile([SQ, 2, 512], F32, tag="psB")
                                for sc in range(2):
                                    nc.tensor.matmul(
                                        scp[:, sc, :],
                                        lhsT=hT[po:po + 64, hp, b, :],
                                        rhs=kTb[po:po + 64, hp,
                                                sc * 512:(sc + 1) * 512],
                                        start=True, stop=True)
                                sume = st_pool.tile([SQ, 1], F32, tag="sume")
                                w_sb = w_pool.tile([SQ, SKV], BF16, tag="w")
                                nc.scalar.activation(
                                    out=w_sb, in_=scp.rearrange("p a s -> p (a s)"),
                                    func=AF.Exp, scale=1.0 / np.sqrt(HD),
                                    accum_out=sume)
                                rec = st_pool.tile([SQ, 1], F32, tag="rec")
                                nc.vector.reciprocal(rec, sume)
                                nc.vector.tensor_scalar_mul(w_sb, w_sb, rec)
                                # transpose w -> wT [s-part, st, qp]
                                wTt = wT_pool.tile([128, DT, SQ], BF16, tag="wT")
                                for g in range(2):
                                    pt = psA.tile([128, 512], BF16, tag="psA")
                                    for j in range(4):
                                        stw = g * 4 + j
                                        nc.tensor.transpose(
                                            pt[:, j * SQ:(j + 1) * SQ],
                                            w_sb[:, stw * 128:(stw + 1) * 128],
                                            id_bf[:SQ, :SQ])
                                    src = pt[:, :4 * SQ].rearrange(
                                        "p (j q) -> p j q", j=4)
                                    if g == 0:
                                        nc.vector.tensor_copy(
                                            out=wTt[:, 0:4, :], in_=src)
                                    else:
                                        nc.scalar.copy(out=wTt[:, 4:8, :], in_=src)
                                wTs.append(wTt)
                            # AV for the pair: out [128, 80] (two heads on partitions)
                            pav = psA.tile([128, 512], F32, tag="psA")
                            for hs in range(2):
                                h = 2 * hp + hs
                                tp = (0, 64) if hs == 1 else None
                                for stv in range(DT):
                                    nc.tensor.matmul(
                                        pav[hs * 64:(hs + 1) * 64, :SQ],
                                        lhsT=vb[:, stv, h * 64:(h + 1) * 64],
                                        rhs=wTs[hs][:, stv, :],
                                        start=(stv == 0), stop=(stv == DT - 1),
                                        tile_position=tp)
                            nc.vector.tensor_copy(out=aT[:, hp, b, :],
                                                  in_=pav[:, :SQ])

                    # ---- attention out-projection (out^T) ----
                    aoT = oT_pool.tile([128, DT, BL, SQ], BF16, tag="oT")
                    Wp_l = Wp[l].rearrange("(t p) n -> p t n", p=128)
                    for nt in range(DT):
                        wpc = wch_pool.tile([128, DT, 128], BF16, tag="wp")
                        stg = stg_pool.tile([128, DT, 128], F32, tag="stg")
                        nc.sync.dma_start(
                            out=stg, in_=Wp_l[:, :, nt * 128:(nt + 1) * 128])
                        nc.gpsimd.tensor_copy(out=wpc, in_=stg)
                        pp = psA.tile([128, 512], F32, tag="psA")
                        for kt in range(DT):
                            nc.tensor.matmul(
                                pp[:, :BL * SQ],
                                lhsT=wpc[:, kt, :],
                                rhs=aT[:, kt, :, :],
                                start=(kt == 0), stop=(kt == DT - 1))
                        nc.scalar.activation(
                            out=aoT[:, nt, :, :],
                            in_=pp[:, :BL * SQ].rearrange("p (b q) -> p b q", b=BL),
                            func=AF.Identity, bias=bp_sb[:, nt:nt + 1])

                    # ---- back to natural + residual + LN1 + pT ----
                    pT = xT_pool.tile([128, DT, BL, SQ], BF16, tag="xT")
                    for b in range(BL):
                        for nt in range(DT):
                            pt = psA.tile([128, 512], BF16, tag="psA")
                            nc.tensor.transpose(pt[:SQ, :128], aoT[:, nt, b, :],
                                                id_bf[:128, :128])
                            nc.vector.tensor_add(
                                xs[b][:, nt * 128:(nt + 1) * 128],
                                xs[b][:, nt * 128:(nt + 1) * 128], pt[:SQ, :128])
                        ln(xs[b], g1_bc, b1_bc)
                        transpose_nat_to_T(xs[b], pT, b, tt_pool)

                    # ---- ffn in (out^T) + gelu ----
                    gT = gT_pool.tile([128, FT, BL, SQ], BF16, tag="gT")
                    Wf_l = Wf[l].rearrange("(t p) n -> p t n", p=128)
                    for nt in range(FT):
                        wfc = wch_pool.tile([128, DT, 128], BF16, tag="wf")
                        stg = stg_pool.tile([128, DT, 128], F32, tag="stg")
                        nc.sync.dma_start(
                            out=stg, in_=Wf_l[:, :, nt * 128:(nt + 1) * 128])
                        if nt % 2 == 0:
                            nc.vector.tensor_copy(out=wfc, in_=stg)
                        else:
                            nc.gpsimd.tensor_copy(out=wfc, in_=stg)
                        pf = psA.tile([128, 512], F32, tag="psA")
                        for kt in range(DT):
                            nc.tensor.matmul(
                                pf[:, :BL * SQ],
                                lhsT=wfc[:, kt, :],
                                rhs=pT[:, kt, :, :],
                                start=(kt == 0), stop=(kt == DT - 1))
                        if GELU_LUT:
                            nc.scalar.activation(
                                out=gT[:, nt, :, :].rearrange("p b q -> p (b q)"),
                                in_=pf[:, :BL * SQ], func=AF.Gelu_apprx_tanh,
                                bias=bf_sb[:, nt:nt + 1])
                        else:
                            # gT = (tanh(GELU_S*(t + GELU_C t^3)) + 1)*t, t=x+b
                            xg = gel_pool.tile([128, BL * SQ], F32, tag="gx")
                            nc.scalar.activation(out=xg, in_=pf[:, :BL * SQ],
                                                 func=AF.Identity,
                                                 bias=bf_sb[:, nt:nt + 1])
                            u = gel_pool.tile([128, BL * SQ], F32, tag="gu")
                            nc.vector.tensor_mul(u, xg, xg)
                            nc.vector.tensor_mul(u, u, xg)
                            nc.vector.scalar_tensor_tensor(
                                out=u, in0=u, scalar=GELU_C, in1=xg,
                                op0=ALU.mult, op1=ALU.add)
                            nc.scalar.activation(out=u, in_=u, func=AF.Tanh,
                                                 scale=GELU_S)
                            nc.vector.scalar_tensor_tensor(
                                out=gT[:, nt, :, :].rearrange("p b q -> p (b q)"),
                                in0=u, scalar=1.0, in1=xg,
                                op0=ALU.add, op1=ALU.mult)

                    # ---- ffn out (out^T), 0.5 folded into Wm cast ----
                    moT = oT_pool.tile([128, DT, BL, SQ], BF16, tag="oT")
                    Wm_l = Wm[l].rearrange("(t p) n -> p t n", p=128)  # [128,32,D]
                    for nt in range(DT):
                        wmc = wch_pool.tile([128, FT, 128], BF16, tag="wm")
                        for q in range(4):
                            stg = stg_pool.tile([128, DT, 128], F32, tag="stg")
                            nc.sync.dma_start(
                                out=stg,
                                in_=Wm_l[:, 8 * q:8 * (q + 1),
                                         nt * 128:(nt + 1) * 128])
                            nc.scalar.activation(
                                out=wmc[:, 8 * q:8 * (q + 1), :], in_=stg,
                                func=AF.Copy, scale=1.0 if GELU_LUT else 0.5)
                        pm = psA.tile([128, 512], F32, tag="psA")
                        for kt in range(FT):
                            nc.tensor.matmul(
                                pm[:, :BL * SQ],
                                lhsT=wmc[:, kt, :],
                                rhs=gT[:, kt, :, :],
                                start=(kt == 0), stop=(kt == FT - 1))
                        nc.scalar.activation(
                            out=moT[:, nt, :, :],
                            in_=pm[:, :BL * SQ].rearrange("p (b q) -> p b q", b=BL),
                            func=AF.Identity, bias=bm_sb[:, nt:nt + 1])

                    # ---- natural + residual + LN2 + hT for next layer ----
                    if l < L - 1:
                        hT = xT_pool.tile([128, DT, BL, SQ], BF16, tag="xT")
                    for b in range(BL):
                        for nt in range(DT):
                            pt = psA.tile([128, 512], BF16, tag="psA")
                            nc.tensor.transpose(pt[:SQ, :128], moT[:, nt, b, :],
                                                id_bf[:128, :128])
                            nc.vector.tensor_add(
                                xs[b][:, nt * 128:(nt + 1) * 128],
                                xs[b][:, nt * 128:(nt + 1) * 128], pt[:SQ, :128])
                        ln(xs[b], g2_bc, b2_bc)
                        if l < L - 1:
                            transpose_nat_to_T(xs[b], hT, b, tt_pool)
                        else:
                            nc.sync.dma_start(out=out_ext[b], in_=xs[b])

    return nc


_CACHE = {}


def kernel(**inputs):
    if "nc" not in _CACHE:
        _CACHE["nc"] = build_nc()
    nc = _CACHE["nc"]

    x = np.ascontiguousarray(inputs["input_ids"], dtype=np.float32)
    know = np.ascontiguousarray(inputs["input_ids_know"], dtype=np.float32)
    shared = {
        "pos_embed": np.ascontiguousarray(inputs["pos_embed"], np.float32),
        "W_attn": np.ascontiguousarray(inputs["W_attn"], np.float32)[:L],
        "b_attn": np.ascontiguousarray(inputs["b_attn"], np.float32)[:L],
        "W_proj_attn": np.ascontiguousarray(inputs["W_proj_attn"], np.float32)[:L],
        "b_proj_attn": np.ascontiguousarray(inputs["b_proj_attn"], np.float32)[:L],
        "ln1_g": np.ascontiguousarray(inputs["ln1_g"], np.float32)[:L],
        "ln1_b": np.ascontiguousarray(inputs["ln1_b"], np.float32)[:L],
        "W_fc": np.ascontiguousarray(inputs["W_fc"], np.float32)[:L],
        "b_fc": np.ascontiguousarray(inputs["b_fc"], np.float32)[:L],
        "W_proj_mlp": np.ascontiguousarray(inputs["W_proj_mlp"], np.float32)[:L],
        "b_proj_mlp": np.ascontiguousarray(inputs["b_proj_mlp"], np.float32)[:L],
        "ln2_g": np.ascontiguousarray(inputs["ln2_g"], np.float32)[:L],
        "ln2_b": np.ascontiguousarray(inputs["ln2_b"], np.float32)[:L],
    }
    in_maps = []
    for i in range(N_CORES):
        m = dict(shared)
        m["input_ids"] = x[i * BL:(i + 1) * BL]
        m["input_ids_know"] = know[i * BL:(i + 1) * BL]
        in_maps.append(m)

    res = run_bass_kernel_spmd(nc, in_maps, list(range(N_CORES)))
    out = np.concatenate([res.results[i]["out"] for i in range(N_CORES)], axis=0)
    return out.astype(np.float32)



# revision 8
# speedup vs baseline: 1.3403x; 1.3403x over previous
"""Trainium2 Bass kernel for nn_ReasonerModel (12-layer cross-attn transformer).

Sharding: data-parallel over batch. 32 batch elems / 8 cores = 4 per core.
Each core streams the full weights (host-precast bf16, pre-tiled layouts)
and computes its 4 batch rows end-to-end; no collectives.

v2 design: everything lives in TRANSPOSED space (features on partitions,
tokens on the free axis) - zero PE transposes.
  xT      [128, 8, 4, 80] f32   residual stream (d on partitions)
  hbf     [128, 8, 4, 80] bf16  bf16 cast feeding matmuls (q, then p)
  know_b  [128, 8, 1024] bf16   d-on-partitions know, streamed per (l,b)
  kT_b    [128, 8, 1024] bf16   K^T per b (n on partitions, s free)
  vb      [128, 8, 1024] bf16   V per b (s on partitions, n free)
  wT_b    [128, 8, 16, 80] bf16 exp(scores^T) (s on partitions)
  aT      [128, 8, 4, 80] bf16  attention out (n on partitions)
  gT      [128, 32, 4, 80] bf16 gelu(fc) (4D-features on partitions)
Attention computes scores TRANSPOSED directly (lhsT = k-chunk, rhs = q),
softmax denominators via ones-vector matmuls, and folds 1/sum into the
AV psum drain using DMA-broadcast reciprocals (DRAM bounce).
LayerNorm stats (sum x, sum x^2) via ones-vector matmuls over partitions;
mu/rstd broadcast back via DRAM bounce; apply fully in transposed space.
"""

import os
import sys

sys.path.insert(0, "/opt/trn_rl_repo")

import numpy as np

import concourse.bass as bass
import concourse.tile as tile
from concourse import mybir
from concourse.bass_utils import run_bass_kernel_spmd
from concourse.vector_clock import ScopedClock

# model dims (fixed by the problem)
B, SQ, SKV, D, H = 32, 80, 1024, 1024, 16
L = int(os.environ.get("KERNEL_LAYERS", "12"))
HD = D // H          # 64
N_CORES = 8
BL = B // N_CORES    # 4 batch rows per core
DT = D // 128        # 8 d-tiles
FT = 4 * D // 128    # 32 ffn tiles
BQ = BL * SQ         # 320
EPS = 1e-5
SCALE = 1.0 / np.sqrt(HD)

F32 = mybir.dt.float32
BF16 = mybir.dt.bfloat16
AF = mybir.ActivationFunctionType
ALU = mybir.AluOpType


class PatchedTC(tile.TileContext):
    """This container's walrus accepts at most ONE sem wait per instruction;
    Tile may attach several. Peel extras onto preceding same-engine no-ops."""

    def _commit_instruction(self, inst, lazy_reg_writes: bool = True):
        si = getattr(inst, "sync_info", None)
        if (
            si is not None
            and si.on_wait
            and len(si.on_wait) > 1
            and inst.engine != mybir.EngineType.Unassigned
        ):
            waits = list(si.on_wait)
            si.on_wait = [waits[-1]]
            for j, w in enumerate(waits[:-1]):
                nop = mybir.InstNoOp(
                    name=f"{inst.name}-sw{j}",
                    sync_info=mybir.SyncInfo(on_wait=[w], on_update=[]),
                    bass_nofuse=True,
                    engine=inst.engine,
                )
                super()._commit_instruction(nop, lazy_reg_writes=False)
        return super()._commit_instruction(inst, lazy_reg_writes)

    def _drain_and_barrier(self, tick_clock, wait_clock):
        drain_inst = self.nc.sync.drain()
        wait_clock.add_sem_waits(
            drain_inst.ins, ScopedClock({None: tick_clock.global_clock})
        )
        si = drain_inst.ins.sync_info
        if si is not None and si.on_wait and len(si.on_wait) > 1:
            waits = list(si.on_wait)
            si.on_wait = waits[:1]
            for w in waits[1:]:
                extra = self.nc.sync.drain()
                nsi = extra.ins.sync_info
                if nsi is None:
                    extra.ins.sync_info = mybir.SyncInfo(on_wait=[w], on_update=[])
                else:
                    nsi.on_wait = [w]
        self.nc.all_engine_barrier()
        assert self.sems is not None
        popped = self.nc._tile_sem_poison_stack.pop()
        assert popped is self._sem_poison
        self.nc.clear_and_free_semaphores(list(self.sems.allocated().values()))
        self.nc.all_engine_barrier()


def bcast_ap(ap_1d, p):
    """Partition-broadcast a 1-D DRAM AP to [p, n] (stride-0 partition dim)."""
    return bass.AP(
        tensor=ap_1d.tensor, offset=ap_1d.offset, ap=[[0, p]] + list(ap_1d.ap)
    )


def build_nc():
    try:  # lift the stale 192KB/partition SBUF cap to the real usable 208KB
        from concourse import tile_utils

        tile_utils.max_sbuf_usage = 208 * 1024
    except Exception:
        pass

    nc = bass.Bass("TRN2", target_bir_lowering=False, debug=False,
                   num_devices=N_CORES)

    # ---- DRAM I/O (host-prepped layouts; see _prep() below) ----
    xT_in = nc.dram_tensor("xT0", [128, DT, BL, SQ], F32, kind="ExternalInput")
    knowT = nc.dram_tensor("knowT", [BL, 128, DT, SKV], BF16,
                           kind="ExternalInput")
    Wk = nc.dram_tensor("Wk", [L, DT, 128, DT, 128], BF16, kind="ExternalInput")
    Wv = nc.dram_tensor("Wv", [L, 128, DT, D], BF16, kind="ExternalInput")
    Wp = nc.dram_tensor("Wp", [L, DT, 128, DT, 128], BF16, kind="ExternalInput")
    Wf = nc.dram_tensor("Wf", [L, FT, 128, DT, 128], BF16, kind="ExternalInput")
    Wm = nc.dram_tensor("Wm", [L, DT, 128, FT, 128], BF16, kind="ExternalInput")
    bk = nc.dram_tensor("bk", [L, 128, DT], F32, kind="ExternalInput")
    bv = nc.dram_tensor("bv", [L, D], BF16, kind="ExternalInput")
    bp = nc.dram_tensor("bp", [L, 128, DT], F32, kind="ExternalInput")
    bf = nc.dram_tensor("bf", [L, 128, FT], F32, kind="ExternalInput")
    bm = nc.dram_tensor("bm", [L, 128, DT], F32, kind="ExternalInput")
    g1 = nc.dram_tensor("g1", [L, 128, DT], F32, kind="ExternalInput")
    b1 = nc.dram_tensor("b1", [L, 128, DT], F32, kind="ExternalInput")
    g2 = nc.dram_tensor("g2", [L, 128, DT], F32, kind="ExternalInput")
    b2 = nc.dram_tensor("b2", [L, 128, DT], F32, kind="ExternalInput")
    out_ext = nc.dram_tensor("out", [128, DT, BL, SQ], F32,
                             kind="ExternalOutput")

    # DRAM bounce buffers for partition-broadcasts
    sums_dram = nc.dram_tensor("sums_d", [BL, H * SQ], F32)
    ln_dram = nc.dram_tensor("ln_d", [2, 2 * BQ], F32)

    with PatchedTC(nc) as tc:
        import contextlib

        ctx = contextlib.ExitStack()
        with ctx:
            P = lambda **kw: ctx.enter_context(tc.tile_pool(**kw))
            singles = P(name="singles", bufs=1)
            know_pool = P(name="know", bufs=2)
            kv_pool = P(name="kv", bufs=1)       # kT_b + vb
            wT_pool = P(name="wT", bufs=1)
            wkv_pool = P(name="wkv", bufs=1)
            wch_pool = P(name="wch", bufs=2)     # wp/wf chunks
            wm_pool = P(name="wm", bufs=2)       # wm chunks (bigger)
            bc_pool = P(name="bc", bufs=2)       # broadcast tiles
            lbc_pool = P(name="lbc", bufs=1)     # LN broadcast (serial)
            sb_pool = P(name="sb", bufs=2)       # per-layer small biases
            stA_pool = P(name="stA", bufs=1)     # LN tiny stats
            stB_pool = P(name="stB", bufs=2)     # softmax recip tiles
            sq_pool = P(name="sq", bufs=2)       # x^2 / LN scratch
            psA = P(name="psA", bufs=2, space="PSUM")  # [128,512] kv/proj/fc/mlp
            psS = P(name="psS", bufs=2, space="PSUM")  # [128,4,80] scoresT
            psV = P(name="psV", bufs=2, space="PSUM")  # [128,160] AV
            psM = P(name="psM", bufs=2, space="PSUM")  # [1,*] sums/LN stats

            # ---- constants ----
            ones_bf = singles.tile([128, 1], BF16)
            nc.vector.memset(ones_bf, 1.0)
            ones_f32 = singles.tile([128, 1], F32)
            nc.vector.memset(ones_f32, 1.0)
            eps_t = singles.tile([1, 1], F32)
            nc.vector.memset(eps_t, EPS)

            # ---- persistent activations ----
            xT = singles.tile([128, DT, BL, SQ], F32, tag="xT")
            nc.sync.dma_start(out=xT, in_=xT_in[:, :, :, :])
            # hbf holds the bf16 cast of the residual: q before attention,
            # then p (LN1 out) for the MLP, then LN2 out = next layer's q.
            hbf = singles.tile([128, DT, BL, SQ], BF16, tag="hbf")
            for dt in range(DT):
                nc.vector.tensor_copy(out=hbf[:, dt], in_=xT[:, dt])

            aT = singles.tile([128, DT, BL, SQ], BF16, tag="aT")
            gT = singles.tile([128, FT, BL, SQ], BF16, tag="gT")

            def layer_norm(which, g_sb, b_sb):
                """LN over the partition(d) axis of xT; writes xT (f32,
                in-place) and the bf16 cast into hbf."""
                ps_s = psM.tile([1, BQ], F32, tag="psM")
                ps_q = psM.tile([1, BQ], F32, tag="psM")
                for dt in range(DT):
                    x2 = xT[:, dt].rearrange("p b q -> p (b q)")
                    xsq = sq_pool.tile([128, BQ], F32, tag="lns")
                    nc.scalar.activation(out=xsq, in_=x2, func=AF.Square)
                    nc.tensor.matmul(
                        ps_s, lhsT=ones_f32, rhs=x2,
                        start=(dt == 0), stop=(dt == DT - 1))
                    nc.tensor.matmul(
                        ps_q, lhsT=ones_f32, rhs=xsq,
                        start=(dt == 0), stop=(dt == DT - 1))
                # mu = ps_s/D ; var = ps_q/D - mu^2 ; rstd = 1/sqrt(var+eps)
                mu = stA_pool.tile([1, BQ], F32, tag="mu")
                nc.vector.tensor_scalar_mul(mu, ps_s, 1.0 / D)
                musq = stA_pool.tile([1, BQ], F32, tag="musq")
                nc.vector.tensor_tensor(musq, mu, mu, ALU.mult)
                var = stA_pool.tile([1, BQ], F32, tag="var")
                nc.vector.scalar_tensor_tensor(
                    out=var, in0=ps_q, scalar=1.0 / D, in1=musq,
                    op0=ALU.mult, op1=ALU.subtract)
                # rstd = exp(-0.5*ln(var+eps))  (Reciprocal/Rsqrt LUTs are
                # unavailable in this container's walrus)
                lnv = stA_pool.tile([1, BQ], F32, tag="lnv")
                nc.scalar.activation(lnv, var, AF.Ln, bias=eps_t)
                rstd = stA_pool.tile([1, BQ], F32, tag="rstd")
                nc.scalar.activation(rstd, lnv, AF.Exp, scale=-0.5)
                # DRAM bounce -> broadcast [128, 2*BQ]
                nc.sync.dma_start(out=ln_dram[which:which + 1, 0:BQ], in_=mu)
                nc.sync.dma_start(out=ln_dram[which:which + 1, BQ:2 * BQ],
                                  in_=rstd)
                lbc = lbc_pool.tile([128, 2 * BQ], F32, tag="lbc")
                nc.gpsimd.dma_start(out=lbc, in_=bcast_ap(ln_dram[which], 128))
                # apply: x = (x - mu)*rstd*g + b ; hbf = bf16(x)
                for dt in range(DT):
                    x2 = xT[:, dt].rearrange("p b q -> p (b q)")
                    t = sq_pool.tile([128, BQ], F32, tag="lns")
                    nc.vector.tensor_tensor(t, x2, lbc[:, 0:BQ], ALU.subtract)
                    nc.vector.tensor_tensor(t, t, lbc[:, BQ:2 * BQ], ALU.mult)
                    nc.vector.tensor_scalar(
                        x2, t, g_sb[:, dt:dt + 1], b_sb[:, dt:dt + 1],
                        op0=ALU.mult, op1=ALU.add)
                    h2 = hbf[:, dt].rearrange("p b q -> p (b q)")
                    if dt % 2 == 0:
                        nc.scalar.copy(out=h2, in_=x2)
                    else:
                        nc.vector.tensor_copy(out=h2, in_=x2)

            # ================= layers =================
            for l in range(L):
                # ---- per-layer weights / biases ----
                wk_sb = wkv_pool.tile([128, DT, DT, 128], BF16, tag="wk")
                nc.sync.dma_start(
                    out=wk_sb, in_=Wk[l].rearrange("t p d n -> p t d n"))
                wv_sb = wkv_pool.tile([128, DT, D], BF16, tag="wv")
                nc.sync.dma_start(out=wv_sb, in_=Wv[l])
                bk_sb = sb_pool.tile([128, DT], F32, tag="bk")
                nc.sync.dma_start(out=bk_sb, in_=bk[l])
                bp_sb = sb_pool.tile([128, DT], F32, tag="bp")
                nc.sync.dma_start(out=bp_sb, in_=bp[l])
                bm_sb = sb_pool.tile([128, DT], F32, tag="bm")
                nc.sync.dma_start(out=bm_sb, in_=bm[l])
                bf_sb = sb_pool.tile([128, FT], F32, tag="bf")
                nc.sync.dma_start(out=bf_sb, in_=bf[l])
                g1_sb = sb_pool.tile([128, DT], F32, tag="g1")
                nc.sync.dma_start(out=g1_sb, in_=g1[l])
                b1_sb = sb_pool.tile([128, DT], F32, tag="b1")
                nc.sync.dma_start(out=b1_sb, in_=b1[l])
                g2_sb = sb_pool.tile([128, DT], F32, tag="g2")
                nc.sync.dma_start(out=g2_sb, in_=g2[l])
                b2_sb = sb_pool.tile([128, DT], F32, tag="b2")
                nc.sync.dma_start(out=b2_sb, in_=b2[l])
                bv_bc = bc_pool.tile([128, D], BF16, tag="bv")
                nc.gpsimd.dma_start(out=bv_bc, in_=bcast_ap(bv[l], 128))

                # ---- per-batch kv + attention ----
                for b in range(BL):
                    know_b = know_pool.tile([128, DT, SKV], BF16, tag="know")
                    nc.sync.dma_start(out=know_b, in_=knowT[b])

                    # K^T: [n-part, s]
                    kTb = kv_pool.tile([128, DT, SKV], BF16, tag="kT")
                    for nt in range(DT):
                        for sc in range(2):
                            ps = psA.tile([128, 512], F32, tag="psA")
                            for dt in range(DT):
                                nc.tensor.matmul(
                                    ps, lhsT=wk_sb[:, nt, dt],
                                    rhs=know_b[:, dt, sc * 512:(sc + 1) * 512],
                                    start=(dt == 0), stop=(dt == DT - 1))
                            nc.scalar.activation(
                                out=kTb[:, nt, sc * 512:(sc + 1) * 512],
                                in_=ps, func=AF.Identity,
                                bias=bk_sb[:, nt:nt + 1])

                    # V: [s-part, n]
                    vb = kv_pool.tile([128, DT, D], BF16, tag="v")
                    for sv in range(DT):
                        for nh in range(2):
                            ps = psA.tile([128, 512], F32, tag="psA")
                            for dt in range(DT):
                                nc.tensor.matmul(
                                    ps,
                                    lhsT=know_b[:, dt, sv * 128:(sv + 1) * 128],
                                    rhs=wv_sb[:, dt, nh * 512:(nh + 1) * 512],
                                    start=(dt == 0), stop=(dt == DT - 1))
                            nc.vector.tensor_tensor(
                                vb[:, sv, nh * 512:(nh + 1) * 512], ps,
                                bv_bc[:, nh * 512:(nh + 1) * 512], ALU.add)

                    # scores^T + exp: wT_b [s-part, sc, h, q]
                    wTb = wT_pool.tile([128, DT, H, SQ], BF16, tag="wT")
                    for h in range(H):
                        po = (h % 2) * 64
                        hp = h // 2
                        for g in range(2):
                            ps = psS.tile([128, 4, SQ], F32, tag="psS")
                            for j in range(4):
                                sc = g * 4 + j
                                nc.tensor.matmul(
                                    ps[:, j, :],
                                    lhsT=kTb[po:po + 64, hp,
                                             sc * 128:(sc + 1) * 128],
                                    rhs=hbf[po:po + 64, hp, b, :],
                                    start=True, stop=True)
                            nc.scalar.activation(
                                out=wTb[:, g * 4:(g + 1) * 4, h, :],
                                in_=ps, func=AF.Exp, scale=SCALE)

                    # softmax sums per head -> reciprocals -> DRAM bounce
                    for hg in range(4):
                        ps = psM.tile([1, 4 * SQ], F32, tag="psM")
                        for sc in range(DT):
                            nc.tensor.matmul(
                                ps, lhsT=ones_bf,
                                rhs=wTb[:, sc, hg * 4:(hg + 1) * 4, :]
                                .rearrange("p h q -> p (h q)"),
                                start=(sc == 0), stop=(sc == DT - 1))
                        lnp = stB_pool.tile([1, 4 * SQ], F32, tag="lnp")
                        nc.scalar.activation(lnp, ps, AF.Ln)
                        rs = stB_pool.tile([1, 4 * SQ], F32, tag="rs")
                        nc.scalar.activation(rs, lnp, AF.Exp, scale=-1.0)
                        nc.sync.dma_start(
                            out=sums_dram[b:b + 1,
                                          hg * 4 * SQ:(hg + 1) * 4 * SQ],
                            in_=rs)
                    # broadcast: rows 0:64 = even head of pair, 64:128 = odd
                    rs_bc = bc_pool.tile([128, DT, SQ], F32, tag="rsbc")
                    row = sums_dram[b]
                    ap_e = bass.AP(tensor=row.tensor, offset=row.offset,
                                   ap=[[0, 64], [2 * SQ, DT], [1, SQ]])
                    nc.gpsimd.dma_start(out=rs_bc[0:64], in_=ap_e)
                    ap_o = bass.AP(tensor=row.tensor, offset=row.offset + SQ,
                                   ap=[[0, 64], [2 * SQ, DT], [1, SQ]])
                    nc.gpsimd.dma_start(out=rs_bc[64:128], in_=ap_o)

                    # AV (head pairs) + normalize into aT
                    for hp in range(DT):
                        ps = psV.tile([128, 2 * SQ], F32, tag="psV")
                        for sv in range(DT):
                            nc.tensor.matmul(
                                ps,
                                lhsT=vb[:, sv, hp * 128:(hp + 1) * 128],
                                rhs=wTb[:, sv, 2 * hp:2 * hp + 2, :].rearrange(
                                    "p h q -> p (h q)"),
                                start=(sv == 0), stop=(sv == DT - 1))
                        nc.vector.tensor_tensor(
                            aT[0:64, hp, b, :], ps[0:64, 0:SQ],
                            rs_bc[0:64, hp, :], ALU.mult)
                        nc.vector.tensor_tensor(
                            aT[64:128, hp, b, :], ps[64:128, SQ:2 * SQ],
                            rs_bc[64:128, hp, :], ALU.mult)

                # ---- attention out-projection + residual ----
                for nt in range(DT):
                    wpc = wch_pool.tile([128, DT, 128], BF16, tag="wp")
                    nc.sync.dma_start(out=wpc, in_=Wp[l, nt])
                    ps = psA.tile([128, 512], F32, tag="psA")
                    for kt in range(DT):
                        nc.tensor.matmul(
                            ps[:, :BQ], lhsT=wpc[:, kt],
                            rhs=aT[:, kt].rearrange("p b q -> p (b q)"),
                            start=(kt == 0), stop=(kt == DT - 1))
                    nc.vector.scalar_tensor_tensor(
                        out=xT[:, nt].rearrange("p b q -> p (b q)"),
                        in0=ps[:, :BQ], scalar=bp_sb[:, nt:nt + 1],
                        in1=xT[:, nt].rearrange("p b q -> p (b q)"),
                        op0=ALU.add, op1=ALU.add)

                # ---- LN1 -> xT (p) + hbf (bf16 p) ----
                layer_norm(0, g1_sb, b1_sb)

                # ---- ffn in + gelu ----
                for nt in range(FT):
                    wfc = wch_pool.tile([128, DT, 128], BF16, tag="wf")
                    nc.sync.dma_start(out=wfc, in_=Wf[l, nt])
                    ps = psA.tile([128, 512], F32, tag="psA")
                    for kt in range(DT):
                        nc.tensor.matmul(
                            ps[:, :BQ], lhsT=wfc[:, kt],
                            rhs=hbf[:, kt].rearrange("p b q -> p (b q)"),
                            start=(kt == 0), stop=(kt == DT - 1))
                    nc.scalar.activation(
                        out=gT[:, nt].rearrange("p b q -> p (b q)"),
                        in_=ps[:, :BQ], func=AF.Gelu_apprx_tanh,
                        bias=bf_sb[:, nt:nt + 1])

                # ---- ffn out + residual ----
                for nt in range(DT):
                    wmc = wm_pool.tile([128, FT, 128], BF16, tag="wm")
                    nc.sync.dma_start(out=wmc, in_=Wm[l, nt])
                    ps = psA.tile([128, 512], F32, tag="psA")
                    for kt in range(FT):
                        nc.tensor.matmul(
                            ps[:, :BQ], lhsT=wmc[:, kt],
                            rhs=gT[:, kt].rearrange("p b q -> p (b q)"),
                            start=(kt == 0), stop=(kt == FT - 1))
                    nc.vector.scalar_tensor_tensor(
                        out=xT[:, nt].rearrange("p b q -> p (b q)"),
                        in0=ps[:, :BQ], scalar=bm_sb[:, nt:nt + 1],
                        in1=xT[:, nt].rearrange("p b q -> p (b q)"),
                        op0=ALU.add, op1=ALU.add)

                # ---- LN2 -> xT + hbf (next layer's q) ----
                layer_norm(1, g2_sb, b2_sb)

            # epilogue: residual out (transposed; host un-transposes)
            nc.sync.dma_start(out=out_ext[:, :, :, :], in_=xT)

    return nc


_CACHE = {}


def _prep(inputs):
    """Host-side layout/dtype prep. Returns per-core in_maps."""
    import ml_dtypes

    bf16 = ml_dtypes.bfloat16
    f32 = np.float32

    x = np.asarray(inputs["input_ids"], f32) + np.asarray(
        inputs["pos_embed"], f32)[None]
    know = np.asarray(inputs["input_ids_know"], f32)
    Wa = np.asarray(inputs["W_attn"], f32)
    ba = np.asarray(inputs["b_attn"], f32)
    Wpm = np.asarray(inputs["W_proj_attn"], f32)
    bpm = np.asarray(inputs["b_proj_attn"], f32)
    Wfm = np.asarray(inputs["W_fc"], f32)
    bfm = np.asarray(inputs["b_fc"], f32)
    Wmm = np.asarray(inputs["W_proj_mlp"], f32)
    bmm = np.asarray(inputs["b_proj_mlp"], f32)

    def pt(a):  # [L, D'] -> [L, 128, T] with element [l, p, t] = a[l, t*128+p]
        return np.ascontiguousarray(
            a.reshape(L, -1, 128).transpose(0, 2, 1), f32)

    shared = {
        # Wk[l, nt, p, dt, n] = Wa[l, dt*128+p, D + nt*128+n]
        "Wk": np.ascontiguousarray(
            Wa[:L, :, D:2 * D].reshape(L, DT, 128, DT, 128)
            .transpose(0, 3, 2, 1, 4).astype(bf16)),
        # Wv[l, p, dt, n] = Wa[l, dt*128+p, 2D + n]
        "Wv": np.ascontiguousarray(
            Wa[:L, :, 2 * D:].reshape(L, DT, 128, D)
            .transpose(0, 2, 1, 3).astype(bf16)),
        # Wp[l, nt, p, kt, n] = Wp[l, kt*128+p, nt*128+n]
        "Wp": np.ascontiguousarray(
            Wpm[:L].reshape(L, DT, 128, DT, 128)
            .transpose(0, 3, 2, 1, 4).astype(bf16)),
        "Wf": np.ascontiguousarray(
            Wfm[:L].reshape(L, DT, 128, FT, 128)
            .transpose(0, 3, 2, 1, 4).astype(bf16)),
        "Wm": np.ascontiguousarray(
            Wmm[:L].reshape(L, FT, 128, DT, 128)
            .transpose(0, 3, 2, 1, 4).astype(bf16)),
        "bk": pt(ba[:L, D:2 * D]),
        "bv": np.ascontiguousarray(ba[:L, 2 * D:].astype(bf16)),
        "bp": pt(bpm[:L]),
        "bf": pt(bfm[:L]),
        "bm": pt(bmm[:L]),
        "g1": pt(np.asarray(inputs["ln1_g"], f32)[:L]),
        "b1": pt(np.asarray(inputs["ln1_b"], f32)[:L]),
        "g2": pt(np.asarray(inputs["ln2_g"], f32)[:L]),
        "b2": pt(np.asarray(inputs["ln2_b"], f32)[:L]),
    }

    in_maps = []
    for c in range(N_CORES):
        m = dict(shared)
        xs = x[c * BL:(c + 1) * BL]  # [BL, SQ, D]
        # xT[p, dt, b, q] = xs[b, q, dt*128+p]
        m["xT0"] = np.ascontiguousarray(
            xs.reshape(BL, SQ, DT, 128).transpose(3, 2, 0, 1), f32)
        ks = know[c * BL:(c + 1) * BL]  # [BL, SKV, D]
        # knowT[b, p, dt, s] = ks[b, s, dt*128+p]
        m["knowT"] = np.ascontiguousarray(
            ks.reshape(BL, SKV, DT, 128).transpose(0, 3, 2, 1).astype(bf16))
        in_maps.append(m)
    return in_maps


def kernel(**inputs):
    if "nc" not in _CACHE:
        _CACHE["nc"] = build_nc()
    nc = _CACHE["nc"]

    in_maps = _prep(inputs)
    _CACHE["last_in_maps"] = in_maps

    res = run_bass_kernel_spmd(nc, in_maps, list(range(N_CORES)))
    outs = []
    for c in range(N_CORES):
        oT = np.asarray(res.results[c]["out"])  # [128, DT, BL, SQ]
        # out[b, q, dt*128+p] = oT[p, dt, b, q]
        outs.append(oT.transpose(2, 3, 1, 0).reshape(BL, SQ, D))
    return np.ascontiguousarray(np.concatenate(outs, axis=0), np.float32)


# revision 15
# speedup vs baseline: 1.4273x; 1.0649x over previous
"""Trainium2 Bass kernel for nn_ReasonerModel (12-layer cross-attn transformer).

Sharding: data-parallel over batch. 32 batch elems / 8 cores = 4 per core.
Each core streams the full weights (host-precast bf16, pre-tiled layouts)
and computes its 4 batch rows end-to-end; no collectives.

v2 design: everything lives in TRANSPOSED space (features on partitions,
tokens on the free axis) - zero PE transposes.
  xT      [128, 8, 4, 80] f32   residual stream (d on partitions)
  hbf     [128, 8, 4, 80] bf16  bf16 cast feeding matmuls (q, then p)
  know_b  [128, 8, 1024] bf16   d-on-partitions know, streamed per (l,b)
  kT_b    [128, 8, 1024] bf16   K^T per b (n on partitions, s free)
  vb      [128, 8, 1024] bf16   V per b (s on partitions, n free)
  wT_b    [128, 8, 16, 80] bf16 exp(scores^T) (s on partitions)
  aT      [128, 8, 4, 80] bf16  attention out (n on partitions)
  gT      [128, 32, 4, 80] bf16 gelu(fc) (4D-features on partitions)
Attention computes scores TRANSPOSED directly (lhsT = k-chunk, rhs = q),
softmax denominators via ones-vector matmuls, and folds 1/sum into the
AV psum drain using DMA-broadcast reciprocals (DRAM bounce).
LayerNorm stats (sum x, sum x^2) via ones-vector matmuls over partitions;
mu/rstd broadcast back via DRAM bounce; apply fully in transposed space.
"""

import os
import sys

sys.path.insert(0, "/opt/trn_rl_repo")

import numpy as np

import concourse.bass as bass
import concourse.tile as tile
from concourse import mybir
from concourse.bass_utils import run_bass_kernel_spmd
from concourse.vector_clock import ScopedClock

# model dims (fixed by the problem)
B, SQ, SKV, D, H = 32, 80, 1024, 1024, 16
L = int(os.environ.get("KERNEL_LAYERS", "12"))
HD = D // H          # 64
N_CORES = 8
BL = B // N_CORES    # 4 batch rows per core
DT = D // 128        # 8 d-tiles
FT = 4 * D // 128    # 32 ffn tiles
BQ = BL * SQ         # 320
EPS = 1e-5
SCALE = 1.0 / np.sqrt(HD)

F32 = mybir.dt.float32
BF16 = mybir.dt.bfloat16
AF = mybir.ActivationFunctionType
ALU = mybir.AluOpType


class PatchedTC(tile.TileContext):
    """This container's walrus accepts at most ONE sem wait per instruction;
    Tile may attach several. Peel extras onto preceding same-engine no-ops."""

    def _commit_instruction(self, inst, lazy_reg_writes: bool = True):
        si = getattr(inst, "sync_info", None)
        if (
            si is not None
            and si.on_wait
            and len(si.on_wait) > 1
            and inst.engine != mybir.EngineType.Unassigned
        ):
            waits = list(si.on_wait)
            si.on_wait = [waits[-1]]
            for j, w in enumerate(waits[:-1]):
                nop = mybir.InstNoOp(
                    name=f"{inst.name}-sw{j}",
                    sync_info=mybir.SyncInfo(on_wait=[w], on_update=[]),
                    bass_nofuse=True,
                    engine=inst.engine,
                )
                super()._commit_instruction(nop, lazy_reg_writes=False)
        return super()._commit_instruction(inst, lazy_reg_writes)

    def _drain_and_barrier(self, tick_clock, wait_clock):
        drain_inst = self.nc.sync.drain()
        wait_clock.add_sem_waits(
            drain_inst.ins, ScopedClock({None: tick_clock.global_clock})
        )
        si = drain_inst.ins.sync_info
        if si is not None and si.on_wait and len(si.on_wait) > 1:
            waits = list(si.on_wait)
            si.on_wait = waits[:1]
            for w in waits[1:]:
                extra = self.nc.sync.drain()
                nsi = extra.ins.sync_info
                if nsi is None:
                    extra.ins.sync_info = mybir.SyncInfo(on_wait=[w], on_update=[])
                else:
                    nsi.on_wait = [w]
        self.nc.all_engine_barrier()
        assert self.sems is not None
        popped = self.nc._tile_sem_poison_stack.pop()
        assert popped is self._sem_poison
        self.nc.clear_and_free_semaphores(list(self.sems.allocated().values()))
        self.nc.all_engine_barrier()


def bcast_ap(ap_1d, p):
    """Partition-broadcast a 1-D DRAM AP to [p, n] (stride-0 partition dim)."""
    return bass.AP(
        tensor=ap_1d.tensor, offset=ap_1d.offset, ap=[[0, p]] + list(ap_1d.ap)
    )


def build_nc():
    try:  # lift the stale 192KB/partition SBUF cap to the real usable 208KB
        from concourse import tile_utils

        tile_utils.max_sbuf_usage = 208 * 1024
    except Exception:
        pass

    nc = bass.Bass("TRN2", target_bir_lowering=False, debug=False,
                   num_devices=N_CORES)

    # ---- DRAM I/O (host-prepped layouts; see _prep() below) ----
    xT_in = nc.dram_tensor("xT0", [128, DT, BL, SQ], F32, kind="ExternalInput")
    knowT = nc.dram_tensor("knowT", [BL, 128, DT, SKV], BF16,
                           kind="ExternalInput")
    Wk = nc.dram_tensor("Wk", [L, DT, 128, DT, 128], BF16, kind="ExternalInput")
    Wv = nc.dram_tensor("Wv", [L, 128, DT, D], BF16, kind="ExternalInput")
    Wp = nc.dram_tensor("Wp", [L, DT, 128, DT, 128], BF16, kind="ExternalInput")
    Wf = nc.dram_tensor("Wf", [L, FT, 128, DT, 128], BF16, kind="ExternalInput")
    Wm = nc.dram_tensor("Wm", [L, DT, 128, FT, 128], BF16, kind="ExternalInput")
    bk = nc.dram_tensor("bk", [L, 128, DT], F32, kind="ExternalInput")
    bv = nc.dram_tensor("bv", [L, D], BF16, kind="ExternalInput")
    bp = nc.dram_tensor("bp", [L, 128, DT], F32, kind="ExternalInput")
    bf = nc.dram_tensor("bf", [L, 128, FT], F32, kind="ExternalInput")
    bm = nc.dram_tensor("bm", [L, 128, DT], F32, kind="ExternalInput")
    g1 = nc.dram_tensor("g1", [L, 128, DT], F32, kind="ExternalInput")
    b1 = nc.dram_tensor("b1", [L, 128, DT], F32, kind="ExternalInput")
    g2 = nc.dram_tensor("g2", [L, 128, DT], F32, kind="ExternalInput")
    b2 = nc.dram_tensor("b2", [L, 128, DT], F32, kind="ExternalInput")
    out_ext = nc.dram_tensor("out", [128, DT, BL, SQ], F32,
                             kind="ExternalOutput")

    with PatchedTC(nc) as tc:
        import contextlib

        ctx = contextlib.ExitStack()
        with ctx:
            P = lambda **kw: ctx.enter_context(tc.tile_pool(**kw))
            singles = P(name="singles", bufs=1)
            know_pool = P(name="know", bufs=2)
            kv_pool = P(name="kv", bufs=1)       # kT_b + vb
            wT_pool = P(name="wT", bufs=1)
            wkv_pool = P(name="wkv", bufs=1)
            wch_pool = P(name="wch", bufs=2)     # wp/wf chunks
            wm_pool = P(name="wm", bufs=2)       # wm chunks (bigger)
            bc_pool = P(name="bc", bufs=2)       # broadcast tiles
            sb_pool = P(name="sb", bufs=2)       # per-layer small biases
            stA_pool = P(name="stA", bufs=1)     # LN tiny stats
            stB_pool = P(name="stB", bufs=2)     # softmax recip tiles
            sq_pool = P(name="sq", bufs=2)       # x^2 / LN scratch
            psA = P(name="psA", bufs=2, space="PSUM")  # [128,512] kv/proj/fc/mlp
            psS = P(name="psS", bufs=2, space="PSUM")  # [128,4,80] scoresT
            psV = P(name="psV", bufs=2, space="PSUM")  # [128,160] AV
            psM = P(name="psM", bufs=2, space="PSUM")  # [1,*] sums/LN stats

            # ---- constants ----
            ones_bf = singles.tile([128, 1], BF16)
            nc.vector.memset(ones_bf, 1.0)
            ones_f32 = singles.tile([128, 1], F32)
            nc.vector.memset(ones_f32, 1.0)
            ones_row = singles.tile([1, 128], F32)
            nc.vector.memset(ones_row, 1.0)
            eps_t = singles.tile([1, 1], F32)
            nc.vector.memset(eps_t, EPS)

            # ---- persistent activations ----
            xT = singles.tile([128, DT, BL, SQ], F32, tag="xT")
            nc.sync.dma_start(out=xT, in_=xT_in[:, :, :, :])
            # hbf holds the bf16 cast of the residual: q before attention,
            # then p (LN1 out) for the MLP, then LN2 out = next layer's q.
            hbf = singles.tile([128, DT, BL, SQ], BF16, tag="hbf")
            for dt in range(DT):
                nc.vector.tensor_copy(out=hbf[:, dt], in_=xT[:, dt])

            aT = singles.tile([128, DT, BL, SQ], BF16, tag="aT")
            gT = singles.tile([128, FT, BL, SQ], BF16, tag="gT")

            def ln_stats():
                """LN stats over the partition(d) axis of xT; returns psum
                broadcast tiles (mu_bc, rstd_bc) [128, BQ]."""
                ps_s = psM.tile([1, BQ], F32, tag="psM", name="ps_s")
                ps_q = psM.tile([1, BQ], F32, tag="psM", name="ps_q")
                for dt in range(DT):
                    x2 = xT[:, dt].rearrange("p b q -> p (b q)")
                    xsq = sq_pool.tile([128, BQ], F32, tag="lns", name="xsq")
                    nc.scalar.activation(out=xsq, in_=x2, func=AF.Square)
                    nc.tensor.matmul(
                        ps_s, lhsT=ones_f32, rhs=x2,
                        start=(dt == 0), stop=(dt == DT - 1))
                    nc.tensor.matmul(
                        ps_q, lhsT=ones_f32, rhs=xsq,
                        start=(dt == 0), stop=(dt == DT - 1))
                # mu = ps_s/D ; var = ps_q/D - mu^2 ; rstd = 1/sqrt(var+eps)
                mu = stA_pool.tile([1, BQ], F32, tag="mu")
                nc.vector.tensor_scalar_mul(mu, ps_s, 1.0 / D)
                musq = stA_pool.tile([1, BQ], F32, tag="musq")
                nc.vector.tensor_tensor(musq, mu, mu, ALU.mult)
                var = stA_pool.tile([1, BQ], F32, tag="var")
                nc.vector.scalar_tensor_tensor(
                    out=var, in0=ps_q, scalar=1.0 / D, in1=musq,
                    op0=ALU.mult, op1=ALU.subtract)
                # rstd = exp(-0.5*ln(var+eps))  (Reciprocal/Rsqrt LUTs are
                # unavailable in this container's walrus)
                lnv = stA_pool.tile([1, BQ], F32, tag="lnv")
                nc.scalar.activation(lnv, var, AF.Ln, bias=eps_t)
                rstd = stA_pool.tile([1, BQ], F32, tag="rstd")
                nc.scalar.activation(rstd, lnv, AF.Exp, scale=-0.5)
                # PE-broadcast to all partitions (psum-resident)
                mu_bc = psV.tile([128, BQ], F32, tag="psV", name="mu_bc")
                nc.tensor.matmul(mu_bc, lhsT=ones_row, rhs=mu,
                                 start=True, stop=True)
                rstd_bc = psV.tile([128, BQ], F32, tag="psV", name="rstd_bc")
                nc.tensor.matmul(rstd_bc, lhsT=ones_row, rhs=rstd,
                                 start=True, stop=True)
                return mu_bc, rstd_bc

            def ln_apply(mu_bc, rstd_bc, g_sb, b_sb):
                """x = (x - mu)*rstd*g + b ; hbf = bf16(x), chunk by chunk."""
                for dt in range(DT):
                    x2 = xT[:, dt].rearrange("p b q -> p (b q)")
                    t = sq_pool.tile([128, BQ], F32, tag="lns", name="lnt")
                    nc.vector.tensor_tensor(t, x2, mu_bc, ALU.subtract)
                    nc.vector.tensor_tensor(t, t, rstd_bc, ALU.mult)
                    nc.vector.tensor_scalar(
                        x2, t, g_sb[:, dt:dt + 1], b_sb[:, dt:dt + 1],
                        op0=ALU.mult, op1=ALU.add)
                    h2 = hbf[:, dt].rearrange("p b q -> p (b q)")
                    if dt % 2 == 0:
                        nc.scalar.copy(out=h2, in_=x2)
                    else:
                        nc.vector.tensor_copy(out=h2, in_=x2)

            def emit_layer_weights(l):
                """DMA layer-l kv weights + biases; returns handle dict."""
                w = {}
                w["know0"] = know_pool.tile([128, DT, SKV], BF16, tag="know",
                                            name="know0")
                nc.sync.dma_start(out=w["know0"], in_=knowT[0])
                w["wk"] = wkv_pool.tile([128, DT, DT, 128], BF16, tag="wk", name="wk")
                nc.sync.dma_start(
                    out=w["wk"], in_=Wk[l].rearrange("t p d n -> p t d n"))
                w["wv"] = wkv_pool.tile([128, DT, D], BF16, tag="wv", name="wv")
                nc.sync.dma_start(out=w["wv"], in_=Wv[l])
                for nm, src in [("bk", bk), ("bp", bp), ("bm", bm),
                                ("g1", g1), ("b1", b1), ("g2", g2),
                                ("b2", b2)]:
                    w[nm] = sb_pool.tile([128, DT], F32, tag=nm, name=nm)
                    nc.sync.dma_start(out=w[nm], in_=src[l])
                w["bf"] = sb_pool.tile([128, FT], F32, tag="bf", name="bfs")
                nc.sync.dma_start(out=w["bf"], in_=bf[l])
                w["bv"] = bc_pool.tile([128, D], BF16, tag="bv", name="bv")
                nc.gpsimd.dma_start(out=w["bv"], in_=bcast_ap(bv[l], 128))
                return w

            def emit_know(b):
                know_b = know_pool.tile([128, DT, SKV], BF16, tag="know",
                                        name="know_b")
                nc.sync.dma_start(out=know_b, in_=knowT[b])
                return know_b

            def emit_kT(w, know_b):
                """K^T [n-part, s] for one b."""
                kTb = kv_pool.tile([128, DT, SKV], BF16, tag="kT")
                for nt in range(DT):
                    for sc in range(2):
                        ps = psA.tile([128, 512], F32, tag="psA", name="psk")
                        for dt in range(DT):
                            nc.tensor.matmul(
                                ps, lhsT=w["wk"][:, nt, dt],
                                rhs=know_b[:, dt, sc * 512:(sc + 1) * 512],
                                start=(dt == 0), stop=(dt == DT - 1))
                        nc.scalar.activation(
                            out=kTb[:, nt, sc * 512:(sc + 1) * 512],
                            in_=ps, func=AF.Identity,
                            bias=w["bk"][:, nt:nt + 1])
                return kTb

            def emit_V(w, know_b):
                """V [s-part, n] for one b."""
                vb = kv_pool.tile([128, DT, D], BF16, tag="v")
                for sv in range(DT):
                    for nh in range(2):
                        ps = psA.tile([128, 512], F32, tag="psA", name="psv")
                        for dt in range(DT):
                            nc.tensor.matmul(
                                ps,
                                lhsT=know_b[:, dt, sv * 128:(sv + 1) * 128],
                                rhs=w["wv"][:, dt, nh * 512:(nh + 1) * 512],
                                start=(dt == 0), stop=(dt == DT - 1))
                        nc.vector.tensor_tensor(
                            vb[:, sv, nh * 512:(nh + 1) * 512], ps,
                            w["bv"][:, nh * 512:(nh + 1) * 512], ALU.add)
                return vb

            def emit_attention(b, kTb, vb):
                """scores^T -> exp -> sums -> AV, writing aT[:, :, b, :]."""
                wTb = wT_pool.tile([128, DT, H, SQ], BF16, tag="wT")
                for h in range(H):
                    po = (h % 2) * 64
                    hp = h // 2
                    for g in range(2):
                        ps = psS.tile([128, 4, SQ], F32, tag="psS", name="pss")
                        for j in range(4):
                            sc = g * 4 + j
                            nc.tensor.matmul(
                                ps[:, j, :],
                                lhsT=kTb[po:po + 64, hp,
                                         sc * 128:(sc + 1) * 128],
                                rhs=hbf[po:po + 64, hp, b, :],
                                start=True, stop=True)
                        nc.scalar.activation(
                            out=wTb[:, g * 4:(g + 1) * 4, h, :],
                            in_=ps, func=AF.Exp, scale=SCALE)

                # softmax sums per head -> 1/sum -> PE-broadcast -> sbuf bf16
                rs_sb = bc_pool.tile([128, H, SQ], BF16, tag="rssb")
                for hg in range(4):
                    ps = psM.tile([1, 4 * SQ], F32, tag="psM", name="pssum")
                    for sc in range(DT):
                        nc.tensor.matmul(
                            ps, lhsT=ones_bf,
                            rhs=wTb[:, sc, hg * 4:(hg + 1) * 4, :]
                            .rearrange("p h q -> p (h q)"),
                            start=(sc == 0), stop=(sc == DT - 1))
                    lnp = stB_pool.tile([1, 4 * SQ], F32, tag="lnp")
                    nc.scalar.activation(lnp, ps, AF.Ln)
                    rs = stB_pool.tile([1, 4 * SQ], F32, tag="rs")
                    nc.scalar.activation(rs, lnp, AF.Exp, scale=-1.0)
                    bcp = psS.tile([128, 4 * SQ], F32, tag="psS", name="bcp")
                    nc.tensor.matmul(bcp, lhsT=ones_row, rhs=rs,
                                     start=True, stop=True)
                    nc.scalar.copy(
                        out=rs_sb[:, hg * 4:(hg + 1) * 4, :]
                        .rearrange("p h q -> p (h q)"), in_=bcp)

                # AV (head pairs) + normalize into aT
                for hp in range(DT):
                    ps = psV.tile([128, 2 * SQ], F32, tag="psV", name="psav")
                    for sv in range(DT):
                        nc.tensor.matmul(
                            ps,
                            lhsT=vb[:, sv, hp * 128:(hp + 1) * 128],
                            rhs=wTb[:, sv, 2 * hp:2 * hp + 2, :].rearrange(
                                "p h q -> p (h q)"),
                            start=(sv == 0), stop=(sv == DT - 1))
                    nc.vector.tensor_tensor(
                        aT[0:64, hp, b, :], ps[0:64, 0:SQ],
                        rs_sb[0:64, 2 * hp, :], ALU.mult)
                    nc.vector.tensor_tensor(
                        aT[64:128, hp, b, :], ps[64:128, SQ:2 * SQ],
                        rs_sb[64:128, 2 * hp + 1, :], ALU.mult)

            # ================= layers (kv software-pipelined) =================
            w = emit_layer_weights(0)
            kTb = emit_kT(w, w["know0"])
            vb = emit_V(w, w["know0"])
            for l in range(L):
                for b in range(BL):
                    emit_attention(b, kTb, vb)
                    if b + 1 < BL:
                        know_b = emit_know(b + 1)
                        kTb = emit_kT(w, know_b)
                        vb = emit_V(w, know_b)

                # ---- attention out-projection + residual ----
                for nt in range(DT):
                    wpc = wch_pool.tile([128, DT, 128], BF16, tag="wp")
                    nc.sync.dma_start(out=wpc, in_=Wp[l, nt])
                    ps = psA.tile([128, 512], F32, tag="psA", name="psp")
                    for kt in range(DT):
                        nc.tensor.matmul(
                            ps[:, :BQ], lhsT=wpc[:, kt],
                            rhs=aT[:, kt].rearrange("p b q -> p (b q)"),
                            start=(kt == 0), stop=(kt == DT - 1))
                    nc.vector.scalar_tensor_tensor(
                        out=xT[:, nt].rearrange("p b q -> p (b q)"),
                        in0=ps[:, :BQ], scalar=w["bp"][:, nt:nt + 1],
                        in1=xT[:, nt].rearrange("p b q -> p (b q)"),
                        op0=ALU.add, op1=ALU.add)

                # ---- LN1 stats, then next layer's kT (fills the gap) ----
                mu_bc, rstd_bc = ln_stats()
                wn = None
                if l + 1 < L:
                    wn = emit_layer_weights(l + 1)
                    kTb_n = emit_kT(wn, wn["know0"])
                g1s, b1s, g2s, b2s, bfs = (w["g1"], w["b1"], w["g2"],
                                           w["b2"], w["bf"])
                ln_apply(mu_bc, rstd_bc, g1s, b1s)

                # ---- ffn in + gelu ----
                for nt in range(FT):
                    wfc = wch_pool.tile([128, DT, 128], BF16, tag="wf")
                    nc.sync.dma_start(out=wfc, in_=Wf[l, nt])
                    ps = psA.tile([128, 512], F32, tag="psA", name="psf")
                    for kt in range(DT):
                        nc.tensor.matmul(
                            ps[:, :BQ], lhsT=wfc[:, kt],
                            rhs=hbf[:, kt].rearrange("p b q -> p (b q)"),
                            start=(kt == 0), stop=(kt == DT - 1))
                    nc.scalar.activation(
                        out=gT[:, nt].rearrange("p b q -> p (b q)"),
                        in_=ps[:, :BQ], func=AF.Gelu_apprx_tanh,
                        bias=bfs[:, nt:nt + 1])

                # ---- ffn out + residual ----
                for nt in range(DT):
                    wmc = wm_pool.tile([128, FT, 128], BF16, tag="wm")
                    nc.sync.dma_start(out=wmc, in_=Wm[l, nt])
                    ps = psA.tile([128, 512], F32, tag="psA", name="psm")
                    for kt in range(FT):
                        nc.tensor.matmul(
                            ps[:, :BQ], lhsT=wmc[:, kt],
                            rhs=gT[:, kt].rearrange("p b q -> p (b q)"),
                            start=(kt == 0), stop=(kt == FT - 1))
                    nc.vector.scalar_tensor_tensor(
                        out=xT[:, nt].rearrange("p b q -> p (b q)"),
                        in0=ps[:, :BQ], scalar=w["bm"][:, nt:nt + 1],
                        in1=xT[:, nt].rearrange("p b q -> p (b q)"),
                        op0=ALU.add, op1=ALU.add)

                # ---- LN2 stats, then next layer's V (fills the gap) ----
                mu_bc, rstd_bc = ln_stats()
                if l + 1 < L:
                    vb_n = emit_V(wn, wn["know0"])
                    kTb, vb, w = kTb_n, vb_n, wn
                ln_apply(mu_bc, rstd_bc, g2s, b2s)

            # epilogue: residual out (transposed; host un-transposes)
            nc.sync.dma_start(out=out_ext[:, :, :, :], in_=xT)

    return nc


_CACHE = {}


def _prep(inputs):
    """Host-side layout/dtype prep. Returns per-core in_maps."""
    import ml_dtypes

    bf16 = ml_dtypes.bfloat16
    f32 = np.float32

    x = np.asarray(inputs["input_ids"], f32) + np.asarray(
        inputs["pos_embed"], f32)[None]
    know = np.asarray(inputs["input_ids_know"], f32)
    Wa = np.asarray(inputs["W_attn"], f32)
    ba = np.asarray(inputs["b_attn"], f32)
    Wpm = np.asarray(inputs["W_proj_attn"], f32)
    bpm = np.asarray(inputs["b_proj_attn"], f32)
    Wfm = np.asarray(inputs["W_fc"], f32)
    bfm = np.asarray(inputs["b_fc"], f32)
    Wmm = np.asarray(inputs["W_proj_mlp"], f32)
    bmm = np.asarray(inputs["b_proj_mlp"], f32)

    def pt(a):  # [L, D'] -> [L, 128, T] with element [l, p, t] = a[l, t*128+p]
        return np.ascontiguousarray(
            a.reshape(L, -1, 128).transpose(0, 2, 1), f32)

    shared = {
        # Wk[l, nt, p, dt, n] = Wa[l, dt*128+p, D + nt*128+n]
        "Wk": np.ascontiguousarray(
            Wa[:L, :, D:2 * D].reshape(L, DT, 128, DT, 128)
            .transpose(0, 3, 2, 1, 4).astype(bf16)),
        # Wv[l, p, dt, n] = Wa[l, dt*128+p, 2D + n]
        "Wv": np.ascontiguousarray(
            Wa[:L, :, 2 * D:].reshape(L, DT, 128, D)
            .transpose(0, 2, 1, 3).astype(bf16)),
        # Wp[l, nt, p, kt, n] = Wp[l, kt*128+p, nt*128+n]
        "Wp": np.ascontiguousarray(
            Wpm[:L].reshape(L, DT, 128, DT, 128)
            .transpose(0, 3, 2, 1, 4).astype(bf16)),
        "Wf": np.ascontiguousarray(
            Wfm[:L].reshape(L, DT, 128, FT, 128)
            .transpose(0, 3, 2, 1, 4).astype(bf16)),
        "Wm": np.ascontiguousarray(
            Wmm[:L].reshape(L, FT, 128, DT, 128)
            .transpose(0, 3, 2, 1, 4).astype(bf16)),
        "bk": pt(ba[:L, D:2 * D]),
        "bv": np.ascontiguousarray(ba[:L, 2 * D:].astype(bf16)),
        "bp": pt(bpm[:L]),
        "bf": pt(bfm[:L]),
        "bm": pt(bmm[:L]),
        "g1": pt(np.asarray(inputs["ln1_g"], f32)[:L]),
        "b1": pt(np.asarray(inputs["ln1_b"], f32)[:L]),
        "g2": pt(np.asarray(inputs["ln2_g"], f32)[:L]),
        "b2": pt(np.asarray(inputs["ln2_b"], f32)[:L]),
    }

    in_maps = []
    for c in range(N_CORES):
        m = dict(shared)
        xs = x[c * BL:(c + 1) * BL]  # [BL, SQ, D]
        # xT[p, dt, b, q] = xs[b, q, dt*128+p]
        m["xT0"] = np.ascontiguousarray(
            xs.reshape(BL, SQ, DT, 128).transpose(3, 2, 0, 1), f32)
        ks = know[c * BL:(c + 1) * BL]  # [BL, SKV, D]
        # knowT[b, p, dt, s] = ks[b, s, dt*128+p]
        m["knowT"] = np.ascontiguousarray(
            ks.reshape(BL, SKV, DT, 128).transpose(0, 3, 2, 1).astype(bf16))
        in_maps.append(m)
    return in_maps


def kernel(**inputs):
    if "nc" not in _CACHE:
        _CACHE["nc"] = build_nc()
    nc = _CACHE["nc"]

    in_maps = _prep(inputs)
    _CACHE["last_in_maps"] = in_maps

    res = run_bass_kernel_spmd(nc, in_maps, list(range(N_CORES)))
    outs = []
    for c in range(N_CORES):
        oT = np.asarray(res.results[c]["out"])  # [128, DT, BL, SQ]
        # out[b, q, dt*128+p] = oT[p, dt, b, q]
        outs.append(oT.transpose(2, 3, 1, 0).reshape(BL, SQ, D))
    return np.ascontiguousarray(np.concatenate(outs, axis=0), np.float32)


# revision 16
# speedup vs baseline: 1.6560x; 1.1602x over previous
"""Trainium2 Bass kernel for nn_ReasonerModel (12-layer cross-attn transformer).

Sharding: data-parallel over batch. 32 batch elems / 8 cores = 4 per core.
Each core streams the full weights (host-precast bf16, pre-tiled layouts)
and computes its 4 batch rows end-to-end; no collectives.

v2 design: everything lives in TRANSPOSED space (features on partitions,
tokens on the free axis) - zero PE transposes.
  xT      [128, 8, 4, 80] f32   residual stream (d on partitions)
  hbf     [128, 8, 4, 80] bf16  bf16 cast feeding matmuls (q, then p)
  know_b  [128, 8, 1024] bf16   d-on-partitions know, streamed per (l,b)
  kT_b    [128, 8, 1024] bf16   K^T per b (n on partitions, s free)
  vb      [128, 8, 1024] bf16   V per b (s on partitions, n free)
  wT_b    [128, 8, 16, 80] bf16 exp(scores^T) (s on partitions)
  aT      [128, 8, 4, 80] bf16  attention out (n on partitions)
  gT      [128, 32, 4, 80] bf16 gelu(fc) (4D-features on partitions)
Attention computes scores TRANSPOSED directly (lhsT = k-chunk, rhs = q),
softmax denominators via ones-vector matmuls, and folds 1/sum into the
AV psum drain using DMA-broadcast reciprocals (DRAM bounce).
LayerNorm stats (sum x, sum x^2) via ones-vector matmuls over partitions;
mu/rstd broadcast back via DRAM bounce; apply fully in transposed space.
"""

import os
import sys

sys.path.insert(0, "/opt/trn_rl_repo")

import numpy as np

import concourse.bass as bass
import concourse.tile as tile
from concourse import mybir
from concourse.bass_utils import run_bass_kernel_spmd
from concourse.vector_clock import ScopedClock

# model dims (fixed by the problem)
B, SQ, SKV, D, H = 32, 80, 1024, 1024, 16
L = int(os.environ.get("KERNEL_LAYERS", "12"))
HD = D // H          # 64
N_CORES = 8
BL = B // N_CORES    # 4 batch rows per core
DT = D // 128        # 8 d-tiles
FT = 4 * D // 128    # 32 ffn tiles
BQ = BL * SQ         # 320
EPS = 1e-5
SCALE = 1.0 / np.sqrt(HD)

F32 = mybir.dt.float32
BF16 = mybir.dt.bfloat16
FP8 = mybir.dt.float8e4
AF = mybir.ActivationFunctionType
ALU = mybir.AluOpType
FP8_SCALE = 64.0           # host prescales know/Wk/Wv into e4m3 range
FP8_INV = 1.0 / (FP8_SCALE * FP8_SCALE)
DR = mybir.MatmulPerfMode.DoubleRow


class PatchedTC(tile.TileContext):
    """This container's walrus accepts at most ONE sem wait per instruction;
    Tile may attach several. Peel extras onto preceding same-engine no-ops."""

    def _commit_instruction(self, inst, lazy_reg_writes: bool = True):
        si = getattr(inst, "sync_info", None)
        if (
            si is not None
            and si.on_wait
            and len(si.on_wait) > 1
            and inst.engine != mybir.EngineType.Unassigned
        ):
            waits = list(si.on_wait)
            si.on_wait = [waits[-1]]
            for j, w in enumerate(waits[:-1]):
                nop = mybir.InstNoOp(
                    name=f"{inst.name}-sw{j}",
                    sync_info=mybir.SyncInfo(on_wait=[w], on_update=[]),
                    bass_nofuse=True,
                    engine=inst.engine,
                )
                super()._commit_instruction(nop, lazy_reg_writes=False)
        return super()._commit_instruction(inst, lazy_reg_writes)

    def _drain_and_barrier(self, tick_clock, wait_clock):
        drain_inst = self.nc.sync.drain()
        wait_clock.add_sem_waits(
            drain_inst.ins, ScopedClock({None: tick_clock.global_clock})
        )
        si = drain_inst.ins.sync_info
        if si is not None and si.on_wait and len(si.on_wait) > 1:
            waits = list(si.on_wait)
            si.on_wait = waits[:1]
            for w in waits[1:]:
                extra = self.nc.sync.drain()
                nsi = extra.ins.sync_info
                if nsi is None:
                    extra.ins.sync_info = mybir.SyncInfo(on_wait=[w], on_update=[])
                else:
                    nsi.on_wait = [w]
        self.nc.all_engine_barrier()
        assert self.sems is not None
        popped = self.nc._tile_sem_poison_stack.pop()
        assert popped is self._sem_poison
        self.nc.clear_and_free_semaphores(list(self.sems.allocated().values()))
        self.nc.all_engine_barrier()


def bcast_ap(ap_1d, p):
    """Partition-broadcast a 1-D DRAM AP to [p, n] (stride-0 partition dim)."""
    return bass.AP(
        tensor=ap_1d.tensor, offset=ap_1d.offset, ap=[[0, p]] + list(ap_1d.ap)
    )


def build_nc():
    try:  # lift the stale 192KB/partition SBUF cap to the real usable 208KB
        from concourse import tile_utils

        tile_utils.max_sbuf_usage = 208 * 1024
    except Exception:
        pass

    nc = bass.Bass("TRN2", target_bir_lowering=False, debug=False,
                   num_devices=N_CORES)

    # ---- DRAM I/O (host-prepped layouts; see _prep() below) ----
    xT_in = nc.dram_tensor("xT0", [128, DT, BL, SQ], F32, kind="ExternalInput")
    knowT = nc.dram_tensor("knowT", [BL, 128, DT, SKV], FP8,
                           kind="ExternalInput")
    Wk = nc.dram_tensor("Wk", [L, DT, 128, DT, 128], FP8, kind="ExternalInput")
    Wv = nc.dram_tensor("Wv", [L, 128, DT, D], FP8, kind="ExternalInput")
    Wp = nc.dram_tensor("Wp", [L, DT, 128, DT, 128], BF16, kind="ExternalInput")
    Wf = nc.dram_tensor("Wf", [L, FT, 128, DT, 128], BF16, kind="ExternalInput")
    Wm = nc.dram_tensor("Wm", [L, DT, 128, FT, 128], BF16, kind="ExternalInput")
    bk = nc.dram_tensor("bk", [L, 128, DT], F32, kind="ExternalInput")
    bv = nc.dram_tensor("bv", [L, D], BF16, kind="ExternalInput")
    bp = nc.dram_tensor("bp", [L, 128, DT], F32, kind="ExternalInput")
    bf = nc.dram_tensor("bf", [L, 128, FT], F32, kind="ExternalInput")
    bm = nc.dram_tensor("bm", [L, 128, DT], F32, kind="ExternalInput")
    g1 = nc.dram_tensor("g1", [L, 128, DT], F32, kind="ExternalInput")
    b1 = nc.dram_tensor("b1", [L, 128, DT], F32, kind="ExternalInput")
    g2 = nc.dram_tensor("g2", [L, 128, DT], F32, kind="ExternalInput")
    b2 = nc.dram_tensor("b2", [L, 128, DT], F32, kind="ExternalInput")
    out_ext = nc.dram_tensor("out", [128, DT, BL, SQ], F32,
                             kind="ExternalOutput")

    with PatchedTC(nc) as tc:
        import contextlib

        ctx = contextlib.ExitStack()
        with ctx:
            P = lambda **kw: ctx.enter_context(tc.tile_pool(**kw))
            singles = P(name="singles", bufs=1)
            know_pool = P(name="know", bufs=2)
            kv_pool = P(name="kv", bufs=1)       # kT_b + vb
            wT_pool = P(name="wT", bufs=1)
            wkv_pool = P(name="wkv", bufs=1)
            wch_pool = P(name="wch", bufs=2)     # wp/wf chunks
            wm_pool = P(name="wm", bufs=2)       # wm chunks (bigger)
            bc_pool = P(name="bc", bufs=2)       # broadcast tiles
            sb_pool = P(name="sb", bufs=2)       # per-layer small biases
            stA_pool = P(name="stA", bufs=1)     # LN tiny stats
            stB_pool = P(name="stB", bufs=2)     # softmax recip tiles
            sq_pool = P(name="sq", bufs=2)       # x^2 / LN scratch
            psA = P(name="psA", bufs=2, space="PSUM")  # [128,512] kv/proj/fc/mlp
            psS = P(name="psS", bufs=2, space="PSUM")  # [128,4,80] scoresT
            psV = P(name="psV", bufs=2, space="PSUM")  # [128,160] AV
            psM = P(name="psM", bufs=2, space="PSUM")  # [1,*] sums/LN stats

            # ---- constants ----
            ones_bf = singles.tile([128, 1], BF16)
            nc.vector.memset(ones_bf, 1.0)
            ones_f32 = singles.tile([128, 1], F32)
            nc.vector.memset(ones_f32, 1.0)
            ones_row = singles.tile([1, 128], F32)
            nc.vector.memset(ones_row, 1.0)
            eps_t = singles.tile([1, 1], F32)
            nc.vector.memset(eps_t, EPS)

            # ---- persistent activations ----
            xT = singles.tile([128, DT, BL, SQ], F32, tag="xT")
            nc.sync.dma_start(out=xT, in_=xT_in[:, :, :, :])
            # hbf holds the bf16 cast of the residual: q before attention,
            # then p (LN1 out) for the MLP, then LN2 out = next layer's q.
            hbf = singles.tile([128, DT, BL, SQ], BF16, tag="hbf")
            for dt in range(DT):
                nc.vector.tensor_copy(out=hbf[:, dt], in_=xT[:, dt])

            aT = singles.tile([128, DT, BL, SQ], BF16, tag="aT")
            gT = singles.tile([128, FT, BL, SQ], BF16, tag="gT")

            def ln_stats():
                """LN stats over the partition(d) axis of xT; returns psum
                broadcast tiles (mu_bc, rstd_bc) [128, BQ]."""
                ps_s = psM.tile([1, BQ], F32, tag="psM", name="ps_s")
                ps_q = psM.tile([1, BQ], F32, tag="psM", name="ps_q")
                for dt in range(DT):
                    x2 = xT[:, dt].rearrange("p b q -> p (b q)")
                    xsq = sq_pool.tile([128, BQ], F32, tag="lns", name="xsq")
                    nc.scalar.activation(out=xsq, in_=x2, func=AF.Square)
                    nc.tensor.matmul(
                        ps_s, lhsT=ones_f32, rhs=x2,
                        start=(dt == 0), stop=(dt == DT - 1))
                    nc.tensor.matmul(
                        ps_q, lhsT=ones_f32, rhs=xsq,
                        start=(dt == 0), stop=(dt == DT - 1))
                # mu = ps_s/D ; var = ps_q/D - mu^2 ; rstd = 1/sqrt(var+eps)
                mu = stA_pool.tile([1, BQ], F32, tag="mu")
                nc.vector.tensor_scalar_mul(mu, ps_s, 1.0 / D)
                musq = stA_pool.tile([1, BQ], F32, tag="musq")
                nc.vector.tensor_tensor(musq, mu, mu, ALU.mult)
                var = stA_pool.tile([1, BQ], F32, tag="var")
                nc.vector.scalar_tensor_tensor(
                    out=var, in0=ps_q, scalar=1.0 / D, in1=musq,
                    op0=ALU.mult, op1=ALU.subtract)
                # rstd = exp(-0.5*ln(var+eps))  (Reciprocal/Rsqrt LUTs are
                # unavailable in this container's walrus)
                lnv = stA_pool.tile([1, BQ], F32, tag="lnv")
                nc.scalar.activation(lnv, var, AF.Ln, bias=eps_t)
                rstd = stA_pool.tile([1, BQ], F32, tag="rstd")
                nc.scalar.activation(rstd, lnv, AF.Exp, scale=-0.5)
                # PE-broadcast to all partitions (psum-resident)
                mu_bc = psV.tile([128, BQ], F32, tag="psV", name="mu_bc")
                nc.tensor.matmul(mu_bc, lhsT=ones_row, rhs=mu,
                                 start=True, stop=True)
                rstd_bc = psV.tile([128, BQ], F32, tag="psV", name="rstd_bc")
                nc.tensor.matmul(rstd_bc, lhsT=ones_row, rhs=rstd,
                                 start=True, stop=True)
                return mu_bc, rstd_bc

            def ln_apply(mu_bc, rstd_bc, g_sb, b_sb):
                """x = (x - mu)*rstd*g + b ; hbf = bf16(x), chunk by chunk."""
                for dt in range(DT):
                    x2 = xT[:, dt].rearrange("p b q -> p (b q)")
                    t = sq_pool.tile([128, BQ], F32, tag="lns", name="lnt")
                    nc.vector.tensor_tensor(t, x2, mu_bc, ALU.subtract)
                    nc.vector.tensor_tensor(t, t, rstd_bc, ALU.mult)
                    nc.vector.tensor_scalar(
                        x2, t, g_sb[:, dt:dt + 1], b_sb[:, dt:dt + 1],
                        op0=ALU.mult, op1=ALU.add)
                    h2 = hbf[:, dt].rearrange("p b q -> p (b q)")
                    if dt % 2 == 0:
                        nc.scalar.copy(out=h2, in_=x2)
                    else:
                        nc.vector.tensor_copy(out=h2, in_=x2)

            def emit_layer_weights(l):
                """DMA layer-l kv weights + biases; returns handle dict."""
                w = {}
                w["know0"] = know_pool.tile([128, DT, SKV], FP8, tag="know",
                                            name="know0")
                nc.sync.dma_start(out=w["know0"], in_=knowT[0])
                w["wk"] = wkv_pool.tile([128, DT, DT, 128], FP8, tag="wk", name="wk")
                nc.sync.dma_start(
                    out=w["wk"], in_=Wk[l].rearrange("t p d n -> p t d n"))
                w["wv"] = wkv_pool.tile([128, DT, D], FP8, tag="wv", name="wv")
                nc.sync.dma_start(out=w["wv"], in_=Wv[l])
                for nm, src in [("bk", bk), ("bp", bp), ("bm", bm),
                                ("g1", g1), ("b1", b1), ("g2", g2),
                                ("b2", b2)]:
                    w[nm] = sb_pool.tile([128, DT], F32, tag=nm, name=nm)
                    nc.sync.dma_start(out=w[nm], in_=src[l])
                w["bf"] = sb_pool.tile([128, FT], F32, tag="bf", name="bfs")
                nc.sync.dma_start(out=w["bf"], in_=bf[l])
                w["bv"] = bc_pool.tile([128, D], BF16, tag="bv", name="bv")
                nc.gpsimd.dma_start(out=w["bv"], in_=bcast_ap(bv[l], 128))
                return w

            def emit_know(b):
                know_b = know_pool.tile([128, DT, SKV], FP8, tag="know",
                                        name="know_b")
                nc.sync.dma_start(out=know_b, in_=knowT[b])
                return know_b

            def emit_kT(w, know_b):
                """K^T [n-part, s] for one b."""
                kTb = kv_pool.tile([128, DT, SKV], BF16, tag="kT")
                for nt in range(DT):
                    for sc in range(2):
                        ps = psA.tile([128, 512], F32, tag="psA", name="psk")
                        for k2 in range(DT // 2):
                            nc.tensor.matmul(
                                ps, lhsT=w["wk"][:, nt, 2 * k2:2 * k2 + 2, :],
                                rhs=know_b[:, 2 * k2:2 * k2 + 2,
                                           sc * 512:(sc + 1) * 512],
                                start=(k2 == 0), stop=(k2 == DT // 2 - 1),
                                perf_mode=DR)
                        nc.scalar.activation(
                            out=kTb[:, nt, sc * 512:(sc + 1) * 512],
                            in_=ps, func=AF.Identity, scale=FP8_INV,
                            bias=w["bk"][:, nt:nt + 1])
                return kTb

            def emit_V(w, know_b):
                """V [s-part, n] for one b."""
                vb = kv_pool.tile([128, DT, D], BF16, tag="v")
                for sv in range(DT):
                    for nh in range(2):
                        ps = psA.tile([128, 512], F32, tag="psA", name="psv")
                        for k2 in range(DT // 2):
                            nc.tensor.matmul(
                                ps,
                                lhsT=know_b[:, 2 * k2:2 * k2 + 2,
                                            sv * 128:(sv + 1) * 128],
                                rhs=w["wv"][:, 2 * k2:2 * k2 + 2,
                                            nh * 512:(nh + 1) * 512],
                                start=(k2 == 0), stop=(k2 == DT // 2 - 1),
                                perf_mode=DR)
                        nc.vector.scalar_tensor_tensor(
                            out=vb[:, sv, nh * 512:(nh + 1) * 512],
                            in0=ps, scalar=FP8_INV,
                            in1=w["bv"][:, nh * 512:(nh + 1) * 512],
                            op0=ALU.mult, op1=ALU.add)
                return vb

            def emit_attention(b, kTb, vb):
                """scores^T -> exp -> sums -> AV, writing aT[:, :, b, :]."""
                wTb = wT_pool.tile([128, DT, H, SQ], BF16, tag="wT")
                for h in range(H):
                    po = (h % 2) * 64
                    hp = h // 2
                    for g in range(2):
                        ps = psS.tile([128, 4, SQ], F32, tag="psS", name="pss")
                        for j in range(4):
                            sc = g * 4 + j
                            nc.tensor.matmul(
                                ps[:, j, :],
                                lhsT=kTb[po:po + 64, hp,
                                         sc * 128:(sc + 1) * 128],
                                rhs=hbf[po:po + 64, hp, b, :],
                                start=True, stop=True)
                        nc.scalar.activation(
                            out=wTb[:, g * 4:(g + 1) * 4, h, :],
                            in_=ps, func=AF.Exp, scale=SCALE)

                # softmax sums per head -> 1/sum -> PE-broadcast -> sbuf bf16
                rs_sb = bc_pool.tile([128, H, SQ], BF16, tag="rssb")
                for hg in range(4):
                    ps = psM.tile([1, 4 * SQ], F32, tag="psM", name="pssum")
                    for sc in range(DT):
                        nc.tensor.matmul(
                            ps, lhsT=ones_bf,
                            rhs=wTb[:, sc, hg * 4:(hg + 1) * 4, :]
                            .rearrange("p h q -> p (h q)"),
                            start=(sc == 0), stop=(sc == DT - 1))
                    lnp = stB_pool.tile([1, 4 * SQ], F32, tag="lnp")
                    nc.scalar.activation(lnp, ps, AF.Ln)
                    rs = stB_pool.tile([1, 4 * SQ], F32, tag="rs")
                    nc.scalar.activation(rs, lnp, AF.Exp, scale=-1.0)
                    bcp = psS.tile([128, 4 * SQ], F32, tag="psS", name="bcp")
                    nc.tensor.matmul(bcp, lhsT=ones_row, rhs=rs,
                                     start=True, stop=True)
                    nc.scalar.copy(
                        out=rs_sb[:, hg * 4:(hg + 1) * 4, :]
                        .rearrange("p h q -> p (h q)"), in_=bcp)

                # AV (head pairs) + normalize into aT
                for hp in range(DT):
                    ps = psV.tile([128, 2 * SQ], F32, tag="psV", name="psav")
                    for sv in range(DT):
                        nc.tensor.matmul(
                            ps,
                            lhsT=vb[:, sv, hp * 128:(hp + 1) * 128],
                            rhs=wTb[:, sv, 2 * hp:2 * hp + 2, :].rearrange(
                                "p h q -> p (h q)"),
                            start=(sv == 0), stop=(sv == DT - 1))
                    nc.vector.tensor_tensor(
                        aT[0:64, hp, b, :], ps[0:64, 0:SQ],
                        rs_sb[0:64, 2 * hp, :], ALU.mult)
                    nc.vector.tensor_tensor(
                        aT[64:128, hp, b, :], ps[64:128, SQ:2 * SQ],
                        rs_sb[64:128, 2 * hp + 1, :], ALU.mult)

            # ================= layers (kv software-pipelined) =================
            w = emit_layer_weights(0)
            kTb = emit_kT(w, w["know0"])
            vb = emit_V(w, w["know0"])
            for l in range(L):
                for b in range(BL):
                    emit_attention(b, kTb, vb)
                    if b + 1 < BL:
                        know_b = emit_know(b + 1)
                        kTb = emit_kT(w, know_b)
                        vb = emit_V(w, know_b)

                # ---- attention out-projection + residual ----
                for nt in range(DT):
                    wpc = wch_pool.tile([128, DT, 128], BF16, tag="wp")
                    nc.sync.dma_start(out=wpc, in_=Wp[l, nt])
                    ps = psA.tile([128, 512], F32, tag="psA", name="psp")
                    for kt in range(DT):
                        nc.tensor.matmul(
                            ps[:, :BQ], lhsT=wpc[:, kt],
                            rhs=aT[:, kt].rearrange("p b q -> p (b q)"),
                            start=(kt == 0), stop=(kt == DT - 1))
                    nc.vector.scalar_tensor_tensor(
                        out=xT[:, nt].rearrange("p b q -> p (b q)"),
                        in0=ps[:, :BQ], scalar=w["bp"][:, nt:nt + 1],
                        in1=xT[:, nt].rearrange("p b q -> p (b q)"),
                        op0=ALU.add, op1=ALU.add)

                # ---- LN1 stats, then next layer's kT (fills the gap) ----
                mu_bc, rstd_bc = ln_stats()
                wn = None
                if l + 1 < L:
                    wn = emit_layer_weights(l + 1)
                    kTb_n = emit_kT(wn, wn["know0"])
                g1s, b1s, g2s, b2s, bfs = (w["g1"], w["b1"], w["g2"],
                                           w["b2"], w["bf"])
                ln_apply(mu_bc, rstd_bc, g1s, b1s)

                # ---- ffn in + gelu ----
                for nt in range(FT):
                    wfc = wch_pool.tile([128, DT, 128], BF16, tag="wf")
                    nc.sync.dma_start(out=wfc, in_=Wf[l, nt])
                    ps = psA.tile([128, 512], F32, tag="psA", name="psf")
                    for kt in range(DT):
                        nc.tensor.matmul(
                            ps[:, :BQ], lhsT=wfc[:, kt],
                            rhs=hbf[:, kt].rearrange("p b q -> p (b q)"),
                            start=(kt == 0), stop=(kt == DT - 1))
                    nc.scalar.activation(
                        out=gT[:, nt].rearrange("p b q -> p (b q)"),
                        in_=ps[:, :BQ], func=AF.Gelu_apprx_tanh,
                        bias=bfs[:, nt:nt + 1])

                # ---- ffn out + residual ----
                for nt in range(DT):
                    wmc = wm_pool.tile([128, FT, 128], BF16, tag="wm")
                    nc.sync.dma_start(out=wmc, in_=Wm[l, nt])
                    ps = psA.tile([128, 512], F32, tag="psA", name="psm")
                    for kt in range(FT):
                        nc.tensor.matmul(
                            ps[:, :BQ], lhsT=wmc[:, kt],
                            rhs=gT[:, kt].rearrange("p b q -> p (b q)"),
                            start=(kt == 0), stop=(kt == FT - 1))
                    nc.vector.scalar_tensor_tensor(
                        out=xT[:, nt].rearrange("p b q -> p (b q)"),
                        in0=ps[:, :BQ], scalar=w["bm"][:, nt:nt + 1],
                        in1=xT[:, nt].rearrange("p b q -> p (b q)"),
                        op0=ALU.add, op1=ALU.add)

                # ---- LN2 stats, then next layer's V (fills the gap) ----
                mu_bc, rstd_bc = ln_stats()
                if l + 1 < L:
                    vb_n = emit_V(wn, wn["know0"])
                    kTb, vb, w = kTb_n, vb_n, wn
                ln_apply(mu_bc, rstd_bc, g2s, b2s)

            # epilogue: residual out (transposed; host un-transposes)
            nc.sync.dma_start(out=out_ext[:, :, :, :], in_=xT)

    return nc


_CACHE = {}


def _prep(inputs):
    """Host-side layout/dtype prep. Returns per-core in_maps."""
    import ml_dtypes

    bf16 = ml_dtypes.bfloat16
    fp8 = ml_dtypes.float8_e4m3
    f32 = np.float32

    x = np.asarray(inputs["input_ids"], f32) + np.asarray(
        inputs["pos_embed"], f32)[None]
    know = np.asarray(inputs["input_ids_know"], f32)
    Wa = np.asarray(inputs["W_attn"], f32)
    ba = np.asarray(inputs["b_attn"], f32)
    Wpm = np.asarray(inputs["W_proj_attn"], f32)
    bpm = np.asarray(inputs["b_proj_attn"], f32)
    Wfm = np.asarray(inputs["W_fc"], f32)
    bfm = np.asarray(inputs["b_fc"], f32)
    Wmm = np.asarray(inputs["W_proj_mlp"], f32)
    bmm = np.asarray(inputs["b_proj_mlp"], f32)

    def pt(a):  # [L, D'] -> [L, 128, T] with element [l, p, t] = a[l, t*128+p]
        return np.ascontiguousarray(
            a.reshape(L, -1, 128).transpose(0, 2, 1), f32)

    shared = {
        # Wk[l, nt, p, dt, n] = Wa[l, dt*128+p, D + nt*128+n]  (x64, fp8)
        "Wk": np.ascontiguousarray(
            (Wa[:L, :, D:2 * D] * 64.0).reshape(L, DT, 128, DT, 128)
            .transpose(0, 3, 2, 1, 4).astype(fp8)),
        # Wv[l, p, dt, n] = Wa[l, dt*128+p, 2D + n]  (x64, fp8)
        "Wv": np.ascontiguousarray(
            (Wa[:L, :, 2 * D:] * 64.0).reshape(L, DT, 128, D)
            .transpose(0, 2, 1, 3).astype(fp8)),
        # Wp[l, nt, p, kt, n] = Wp[l, kt*128+p, nt*128+n]
        "Wp": np.ascontiguousarray(
            Wpm[:L].reshape(L, DT, 128, DT, 128)
            .transpose(0, 3, 2, 1, 4).astype(bf16)),
        "Wf": np.ascontiguousarray(
            Wfm[:L].reshape(L, DT, 128, FT, 128)
            .transpose(0, 3, 2, 1, 4).astype(bf16)),
        "Wm": np.ascontiguousarray(
            Wmm[:L].reshape(L, FT, 128, DT, 128)
            .transpose(0, 3, 2, 1, 4).astype(bf16)),
        "bk": pt(ba[:L, D:2 * D]),
        "bv": np.ascontiguousarray(ba[:L, 2 * D:].astype(bf16)),
        "bp": pt(bpm[:L]),
        "bf": pt(bfm[:L]),
        "bm": pt(bmm[:L]),
        "g1": pt(np.asarray(inputs["ln1_g"], f32)[:L]),
        "b1": pt(np.asarray(inputs["ln1_b"], f32)[:L]),
        "g2": pt(np.asarray(inputs["ln2_g"], f32)[:L]),
        "b2": pt(np.asarray(inputs["ln2_b"], f32)[:L]),
    }

    in_maps = []
    for c in range(N_CORES):
        m = dict(shared)
        xs = x[c * BL:(c + 1) * BL]  # [BL, SQ, D]
        # xT[p, dt, b, q] = xs[b, q, dt*128+p]
        m["xT0"] = np.ascontiguousarray(
            xs.reshape(BL, SQ, DT, 128).transpose(3, 2, 0, 1), f32)
        ks = know[c * BL:(c + 1) * BL]  # [BL, SKV, D]
        # knowT[b, p, dt, s] = ks[b, s, dt*128+p]
        m["knowT"] = np.ascontiguousarray(
            (ks * 64.0).reshape(BL, SKV, DT, 128)
            .transpose(0, 3, 2, 1).astype(fp8))
        in_maps.append(m)
    return in_maps


def kernel(**inputs):
    if "nc" not in _CACHE:
        _CACHE["nc"] = build_nc()
    nc = _CACHE["nc"]

    in_maps = _prep(inputs)
    _CACHE["last_in_maps"] = in_maps

    res = run_bass_kernel_spmd(nc, in_maps, list(range(N_CORES)))
    outs = []
    for c in range(N_CORES):
        oT = np.asarray(res.results[c]["out"])  # [128, DT, BL, SQ]
        # out[b, q, dt*128+p] = oT[p, dt, b, q]
        outs.append(oT.transpose(2, 3, 1, 0).reshape(BL, SQ, D))
    return np.ascontiguousarray(np.concatenate(outs, axis=0), np.float32)


# revision 19
# speedup vs baseline: 1.7818x; 1.0760x over previous
"""Trainium2 Bass kernel for nn_ReasonerModel (12-layer cross-attn transformer).

Sharding: data-parallel over batch. 32 batch elems / 8 cores = 4 per core.
Each core streams the full weights (host-precast bf16, pre-tiled layouts)
and computes its 4 batch rows end-to-end; no collectives.

v2 design: everything lives in TRANSPOSED space (features on partitions,
tokens on the free axis) - zero PE transposes.
  xT      [128, 8, 4, 80] f32   residual stream (d on partitions)
  hbf     [128, 8, 4, 80] bf16  bf16 cast feeding matmuls (q, then p)
  know_b  [128, 8, 1024] bf16   d-on-partitions know, streamed per (l,b)
  kT_b    [128, 8, 1024] bf16   K^T per b (n on partitions, s free)
  vb      [128, 8, 1024] bf16   V per b (s on partitions, n free)
  wT_b    [128, 8, 16, 80] bf16 exp(scores^T) (s on partitions)
  aT      [128, 8, 4, 80] bf16  attention out (n on partitions)
  gT      [128, 32, 4, 80] bf16 gelu(fc) (4D-features on partitions)
Attention computes scores TRANSPOSED directly (lhsT = k-chunk, rhs = q),
softmax denominators via ones-vector matmuls, and folds 1/sum into the
AV psum drain using DMA-broadcast reciprocals (DRAM bounce).
LayerNorm stats (sum x, sum x^2) via ones-vector matmuls over partitions;
mu/rstd broadcast back via DRAM bounce; apply fully in transposed space.
"""

import os
import sys

sys.path.insert(0, "/opt/trn_rl_repo")

import numpy as np

import concourse.bass as bass
import concourse.tile as tile
from concourse import mybir
from concourse.bass_utils import run_bass_kernel_spmd
from concourse.vector_clock import ScopedClock

# model dims (fixed by the problem)
B, SQ, SKV, D, H = 32, 80, 1024, 1024, 16
L = int(os.environ.get("KERNEL_LAYERS", "12"))
HD = D // H          # 64
N_CORES = 8
BL = B // N_CORES    # 4 batch rows per core
DT = D // 128        # 8 d-tiles
FT = 4 * D // 128    # 32 ffn tiles
BQ = BL * SQ         # 320
EPS = 1e-5
SCALE = 1.0 / np.sqrt(HD)

F32 = mybir.dt.float32
BF16 = mybir.dt.bfloat16
FP8 = mybir.dt.float8e4
AF = mybir.ActivationFunctionType
ALU = mybir.AluOpType
FP8_SCALE = 64.0           # host prescales know + all weights into e4m3 range
FP8_INV = 1.0 / (FP8_SCALE * FP8_SCALE)
AV_SCALE = 4096.0          # aT carries 4096*a so fp8 stays in normal range
LN4096 = float(np.log(AV_SCALE / FP8_SCALE))
DR = mybir.MatmulPerfMode.DoubleRow


class PatchedTC(tile.TileContext):
    """This container's walrus accepts at most ONE sem wait per instruction;
    Tile may attach several. Peel extras onto preceding same-engine no-ops."""

    def _commit_instruction(self, inst, lazy_reg_writes: bool = True):
        si = getattr(inst, "sync_info", None)
        if (
            si is not None
            and si.on_wait
            and len(si.on_wait) > 1
            and inst.engine != mybir.EngineType.Unassigned
        ):
            waits = list(si.on_wait)
            si.on_wait = [waits[-1]]
            for j, w in enumerate(waits[:-1]):
                nop = mybir.InstNoOp(
                    name=f"{inst.name}-sw{j}",
                    sync_info=mybir.SyncInfo(on_wait=[w], on_update=[]),
                    bass_nofuse=True,
                    engine=inst.engine,
                )
                super()._commit_instruction(nop, lazy_reg_writes=False)
        return super()._commit_instruction(inst, lazy_reg_writes)

    def _drain_and_barrier(self, tick_clock, wait_clock):
        drain_inst = self.nc.sync.drain()
        wait_clock.add_sem_waits(
            drain_inst.ins, ScopedClock({None: tick_clock.global_clock})
        )
        si = drain_inst.ins.sync_info
        if si is not None and si.on_wait and len(si.on_wait) > 1:
            waits = list(si.on_wait)
            si.on_wait = waits[:1]
            for w in waits[1:]:
                extra = self.nc.sync.drain()
                nsi = extra.ins.sync_info
                if nsi is None:
                    extra.ins.sync_info = mybir.SyncInfo(on_wait=[w], on_update=[])
                else:
                    nsi.on_wait = [w]
        self.nc.all_engine_barrier()
        assert self.sems is not None
        popped = self.nc._tile_sem_poison_stack.pop()
        assert popped is self._sem_poison
        self.nc.clear_and_free_semaphores(list(self.sems.allocated().values()))
        self.nc.all_engine_barrier()


def bcast_ap(ap_1d, p):
    """Partition-broadcast a 1-D DRAM AP to [p, n] (stride-0 partition dim)."""
    return bass.AP(
        tensor=ap_1d.tensor, offset=ap_1d.offset, ap=[[0, p]] + list(ap_1d.ap)
    )


def build_nc():
    try:  # lift the stale 192KB/partition SBUF cap to the real usable 208KB
        from concourse import tile_utils

        tile_utils.max_sbuf_usage = 208 * 1024
    except Exception:
        pass

    nc = bass.Bass("TRN2", target_bir_lowering=False, debug=False,
                   num_devices=N_CORES)

    # ---- DRAM I/O (host-prepped layouts; see _prep() below) ----
    xT_in = nc.dram_tensor("xT0", [128, DT, BL, SQ], F32, kind="ExternalInput")
    knowT = nc.dram_tensor("knowT", [BL, 128, DT, SKV], FP8,
                           kind="ExternalInput")
    Wk = nc.dram_tensor("Wk", [L, DT, 128, DT, 128], FP8, kind="ExternalInput")
    Wv = nc.dram_tensor("Wv", [L, 128, DT, D], FP8, kind="ExternalInput")
    Wp = nc.dram_tensor("Wp", [L, DT, 128, DT, 128], FP8, kind="ExternalInput")
    Wf = nc.dram_tensor("Wf", [L, FT, 128, DT, 128], BF16, kind="ExternalInput")
    Wm = nc.dram_tensor("Wm", [L, DT, 128, FT, 128], BF16, kind="ExternalInput")
    bk = nc.dram_tensor("bk", [L, 128, DT], F32, kind="ExternalInput")
    bv = nc.dram_tensor("bv", [L, D], BF16, kind="ExternalInput")
    bp = nc.dram_tensor("bp", [L, 128, DT], F32, kind="ExternalInput")
    bf = nc.dram_tensor("bf", [L, 128, FT], F32, kind="ExternalInput")
    bm = nc.dram_tensor("bm", [L, 128, DT], F32, kind="ExternalInput")
    g1 = nc.dram_tensor("g1", [L, 128, DT], F32, kind="ExternalInput")
    b1 = nc.dram_tensor("b1", [L, 128, DT], F32, kind="ExternalInput")
    g2 = nc.dram_tensor("g2", [L, 128, DT], F32, kind="ExternalInput")
    b2 = nc.dram_tensor("b2", [L, 128, DT], F32, kind="ExternalInput")
    out_ext = nc.dram_tensor("out", [128, DT, BL, SQ], F32,
                             kind="ExternalOutput")

    with PatchedTC(nc) as tc:
        import contextlib

        ctx = contextlib.ExitStack()
        with ctx:
            P = lambda **kw: ctx.enter_context(tc.tile_pool(**kw))
            singles = P(name="singles", bufs=1)
            know_pool = P(name="know", bufs=2)
            kv_pool = P(name="kv", bufs=1)       # kT_b + vb
            wT_pool = P(name="wT", bufs=1)
            wkv_pool = P(name="wkv", bufs=1)
            wch_pool = P(name="wch", bufs=2)     # wp/wf chunks
            wm_pool = P(name="wm", bufs=2)       # wm chunks (bigger)
            bc_pool = P(name="bc", bufs=2)       # broadcast tiles
            sb_pool = P(name="sb", bufs=2)       # per-layer small biases
            stA_pool = P(name="stA", bufs=1)     # LN tiny stats
            stB_pool = P(name="stB", bufs=2)     # softmax recip tiles
            sq_pool = P(name="sq", bufs=2)       # x^2 / LN scratch
            psA = P(name="psA", bufs=3, space="PSUM")  # [128,512] kv/proj/fc/mlp
            psS = P(name="psS", bufs=2, space="PSUM")  # [128,4,80] scoresT
            psV = P(name="psV", bufs=2, space="PSUM")  # [128,160] AV
            psM = P(name="psM", bufs=1, space="PSUM")  # [1,*] sums/LN stats

            # ---- constants ----
            ones_bf = singles.tile([128, 1], BF16)
            nc.vector.memset(ones_bf, 1.0)
            ones_f8 = singles.tile([128, 1], FP8)
            nc.vector.memset(ones_f8, 1.0)
            ones_f32 = singles.tile([128, 1], F32)
            nc.vector.memset(ones_f32, 1.0)
            ones_row = singles.tile([1, 128], F32)
            nc.vector.memset(ones_row, 1.0)
            eps_t = singles.tile([1, 1], F32)
            nc.vector.memset(eps_t, EPS)
            ln4096_t = singles.tile([1, 1], F32)
            nc.vector.memset(ln4096_t, LN4096)

            # ---- persistent activations ----
            xT = singles.tile([128, DT, BL, SQ], F32, tag="xT")
            nc.sync.dma_start(out=xT, in_=xT_in[:, :, :, :])
            # hbf holds the bf16 cast of the residual: q before attention,
            # then p (LN1 out) for the MLP, then LN2 out = next layer's q.
            hbf = singles.tile([128, DT, BL, SQ], BF16, tag="hbf")
            hq8 = singles.tile([128, DT, BL, SQ], FP8, tag="hq8")
            for dt in range(DT):
                nc.vector.tensor_copy(out=hq8[:, dt], in_=xT[:, dt])

            aT = singles.tile([128, DT, BL, SQ], FP8, tag="aT")
            gT = singles.tile([128, FT, BL, SQ], BF16, tag="gT")

            def ln_stats():
                """LN stats over the partition(d) axis of xT; returns psum
                broadcast tiles (mu_bc, rstd_bc) [128, BQ]."""
                ps_s = psM.tile([1, BQ], F32, tag="psM", name="ps_s")
                ps_q = psS.tile([1, BQ], F32, tag="psS", name="ps_q")
                for dt in range(DT):
                    x2 = xT[:, dt].rearrange("p b q -> p (b q)")
                    xsq = sq_pool.tile([128, BQ], F32, tag="lns", name="xsq")
                    nc.scalar.activation(out=xsq, in_=x2, func=AF.Square)
                    nc.tensor.matmul(
                        ps_s, lhsT=ones_f32, rhs=x2,
                        start=(dt == 0), stop=(dt == DT - 1))
                    nc.tensor.matmul(
                        ps_q, lhsT=ones_f32, rhs=xsq,
                        start=(dt == 0), stop=(dt == DT - 1))
                # mu = ps_s/D ; var = ps_q/D - mu^2 ; rstd = 1/sqrt(var+eps)
                mu = stA_pool.tile([1, BQ], F32, tag="mu")
                nc.vector.tensor_scalar_mul(mu, ps_s, 1.0 / D)
                musq = stA_pool.tile([1, BQ], F32, tag="musq")
                nc.vector.tensor_tensor(musq, mu, mu, ALU.mult)
                var = stA_pool.tile([1, BQ], F32, tag="var")
                nc.vector.scalar_tensor_tensor(
                    out=var, in0=ps_q, scalar=1.0 / D, in1=musq,
                    op0=ALU.mult, op1=ALU.subtract)
                # rstd = exp(-0.5*ln(var+eps))  (Reciprocal/Rsqrt LUTs are
                # unavailable in this container's walrus)
                lnv = stA_pool.tile([1, BQ], F32, tag="lnv")
                nc.scalar.activation(lnv, var, AF.Ln, bias=eps_t)
                rstd = stA_pool.tile([1, BQ], F32, tag="rstd")
                nc.scalar.activation(rstd, lnv, AF.Exp, scale=-0.5)
                # PE-broadcast to all partitions (psum-resident)
                mu_bc = psV.tile([128, BQ], F32, tag="psV", name="mu_bc")
                nc.tensor.matmul(mu_bc, lhsT=ones_row, rhs=mu,
                                 start=True, stop=True)
                rstd_bc = psV.tile([128, BQ], F32, tag="psV", name="rstd_bc")
                nc.tensor.matmul(rstd_bc, lhsT=ones_row, rhs=rstd,
                                 start=True, stop=True)
                return mu_bc, rstd_bc

            def ln_apply(mu_bc, rstd_bc, g_sb, b_sb, cast_out):
                """x = (x - mu)*rstd*g + b ; cast_out = lowprec(x)."""
                for dt in range(DT):
                    x2 = xT[:, dt].rearrange("p b q -> p (b q)")
                    t = sq_pool.tile([128, BQ], F32, tag="lns", name="lnt")
                    nc.vector.tensor_tensor(t, x2, mu_bc, ALU.subtract)
                    nc.vector.tensor_tensor(t, t, rstd_bc, ALU.mult)
                    nc.vector.tensor_scalar(
                        x2, t, g_sb[:, dt:dt + 1], b_sb[:, dt:dt + 1],
                        op0=ALU.mult, op1=ALU.add)
                    h2 = cast_out[:, dt].rearrange("p b q -> p (b q)")
                    if dt % 2 == 0:
                        nc.scalar.copy(out=h2, in_=x2)
                    else:
                        nc.vector.tensor_copy(out=h2, in_=x2)

            def emit_layer_weights(l):
                """DMA layer-l kv weights + biases; returns handle dict."""
                w = {}
                w["know0"] = know_pool.tile([128, DT, SKV], FP8, tag="know",
                                            name="know0")
                nc.sync.dma_start(out=w["know0"], in_=knowT[0])
                w["wk"] = wkv_pool.tile([128, DT, DT, 128], FP8, tag="wk", name="wk")
                nc.sync.dma_start(
                    out=w["wk"], in_=Wk[l].rearrange("t p d n -> p t d n"))
                w["wv"] = wkv_pool.tile([128, DT, D], FP8, tag="wv", name="wv")
                nc.sync.dma_start(out=w["wv"], in_=Wv[l])
                for nm, src in [("bk", bk), ("bp", bp), ("bm", bm),
                                ("g1", g1), ("b1", b1), ("g2", g2),
                                ("b2", b2)]:
                    w[nm] = sb_pool.tile([128, DT], F32, tag=nm, name=nm)
                    nc.sync.dma_start(out=w[nm], in_=src[l])
                w["bf"] = sb_pool.tile([128, FT], F32, tag="bf", name="bfs")
                nc.sync.dma_start(out=w["bf"], in_=bf[l])
                w["bv"] = bc_pool.tile([128, D], BF16, tag="bv", name="bv")
                nc.gpsimd.dma_start(out=w["bv"], in_=bcast_ap(bv[l], 128))
                return w

            def emit_know(b):
                know_b = know_pool.tile([128, DT, SKV], FP8, tag="know",
                                        name="know_b")
                nc.sync.dma_start(out=know_b, in_=knowT[b])
                return know_b

            def emit_kT(w, know_b):
                """K^T [n-part, s] for one b."""
                kTb = kv_pool.tile([128, DT, SKV], FP8, tag="kT")
                for nt in range(DT):
                    for sc in range(2):
                        ps = psA.tile([128, 512], F32, tag="psA", name="psk")
                        for k2 in range(DT // 2):
                            nc.tensor.matmul(
                                ps, lhsT=w["wk"][:, nt, 2 * k2:2 * k2 + 2, :],
                                rhs=know_b[:, 2 * k2:2 * k2 + 2,
                                           sc * 512:(sc + 1) * 512],
                                start=(k2 == 0), stop=(k2 == DT // 2 - 1),
                                perf_mode=DR)
                        if sc == 0:
                            nc.scalar.activation(
                                out=kTb[:, nt, sc * 512:(sc + 1) * 512],
                                in_=ps, func=AF.Identity,
                                scale=1.0 / FP8_SCALE,
                                bias=w["bk"][:, nt:nt + 1])
                        else:
                            nc.vector.tensor_scalar(
                                kTb[:, nt, sc * 512:(sc + 1) * 512], ps,
                                1.0 / FP8_SCALE, w["bk"][:, nt:nt + 1],
                                op0=ALU.mult, op1=ALU.add)
                return kTb

            def emit_V(w, know_b):
                """V [s-part, n] for one b."""
                vb = kv_pool.tile([128, DT, D], FP8, tag="v")
                for sv in range(DT):
                    for nh in range(2):
                        ps = psA.tile([128, 512], F32, tag="psA", name="psv")
                        for k2 in range(DT // 2):
                            nc.tensor.matmul(
                                ps,
                                lhsT=know_b[:, 2 * k2:2 * k2 + 2,
                                            sv * 128:(sv + 1) * 128],
                                rhs=w["wv"][:, 2 * k2:2 * k2 + 2,
                                            nh * 512:(nh + 1) * 512],
                                start=(k2 == 0), stop=(k2 == DT // 2 - 1),
                                perf_mode=DR)
                        nc.vector.scalar_tensor_tensor(
                            out=vb[:, sv, nh * 512:(nh + 1) * 512],
                            in0=ps, scalar=1.0 / FP8_SCALE,
                            in1=w["bv"][:, nh * 512:(nh + 1) * 512],
                            op0=ALU.mult, op1=ALU.add)
                return vb

            def emit_attention(b, kTb, vb):
                """scores^T -> exp -> sums -> AV, writing aT[:, :, b, :]."""
                wTb = wT_pool.tile([128, DT, H, SQ], FP8, tag="wT")
                for h in range(H):
                    po = (h % 2) * 64
                    hp = h // 2
                    for g in range(2):
                        ps = psS.tile([128, 4, SQ], F32, tag="psS", name="pss")
                        for j in range(4):
                            sc = g * 4 + j
                            nc.tensor.matmul(
                                ps[:, j, :],
                                lhsT=kTb[po:po + 64, hp,
                                         sc * 128:(sc + 1) * 128],
                                rhs=hq8[po:po + 64, hp, b, :],
                                start=True, stop=True)
                        nc.scalar.activation(
                            out=wTb[:, g * 4:(g + 1) * 4, h, :],
                            in_=ps, func=AF.Exp, scale=SCALE / FP8_SCALE)

                # softmax sums per head -> 1/sum -> PE-broadcast -> sbuf bf16
                rs_sb = bc_pool.tile([128, H, SQ], BF16, tag="rssb")
                for hg in range(4):
                    if hg % 2 == 0:
                        ps = psM.tile([1, 4 * SQ], F32, tag="psM",
                                      name="pssum")
                    else:
                        ps = psS.tile([1, 4 * SQ], F32, tag="psS",
                                      name="pssum2")
                    for sc in range(DT):
                        nc.tensor.matmul(
                            ps, lhsT=ones_f8,
                            rhs=wTb[:, sc, hg * 4:(hg + 1) * 4, :]
                            .rearrange("p h q -> p (h q)"),
                            start=(sc == 0), stop=(sc == DT - 1))
                    lnp = stB_pool.tile([1, 4 * SQ], F32, tag="lnp")
                    nc.scalar.activation(lnp, ps, AF.Ln)
                    rs = stB_pool.tile([1, 4 * SQ], F32, tag="rs")
                    nc.scalar.activation(rs, lnp, AF.Exp, scale=-1.0,
                                         bias=ln4096_t)
                    bcp = psS.tile([128, 4 * SQ], F32, tag="psS", name="bcp")
                    nc.tensor.matmul(bcp, lhsT=ones_row, rhs=rs,
                                     start=True, stop=True)
                    nc.scalar.copy(
                        out=rs_sb[:, hg * 4:(hg + 1) * 4, :]
                        .rearrange("p h q -> p (h q)"), in_=bcp)

                # AV (head pairs) + normalize into aT
                for hp in range(DT):
                    ps = psV.tile([128, 2 * SQ], F32, tag="psV", name="psav")
                    for sv in range(DT):
                        nc.tensor.matmul(
                            ps,
                            lhsT=vb[:, sv, hp * 128:(hp + 1) * 128],
                            rhs=wTb[:, sv, 2 * hp:2 * hp + 2, :].rearrange(
                                "p h q -> p (h q)"),
                            start=(sv == 0), stop=(sv == DT - 1))
                    nc.vector.tensor_tensor(
                        aT[0:64, hp, b, :], ps[0:64, 0:SQ],
                        rs_sb[0:64, 2 * hp, :], ALU.mult)
                    nc.vector.tensor_tensor(
                        aT[64:128, hp, b, :], ps[64:128, SQ:2 * SQ],
                        rs_sb[64:128, 2 * hp + 1, :], ALU.mult)

            # ================= layers (kv software-pipelined) =================
            w = emit_layer_weights(0)
            kTb = emit_kT(w, w["know0"])
            vb = emit_V(w, w["know0"])
            for l in range(L):
                for b in range(BL):
                    emit_attention(b, kTb, vb)
                    if b + 1 < BL:
                        know_b = emit_know(b + 1)
                        kTb = emit_kT(w, know_b)
                        vb = emit_V(w, know_b)

                # ---- attention out-projection + residual ----
                for nt in range(DT):
                    wpc = wch_pool.tile([128, DT, 128], FP8, tag="wp")
                    nc.sync.dma_start(out=wpc, in_=Wp[l, nt])
                    ps = psA.tile([128, 512], F32, tag="psA", name="psp")
                    for k2 in range(DT // 2):
                        nc.tensor.matmul(
                            ps[:, :BQ], lhsT=wpc[:, 2 * k2:2 * k2 + 2, :],
                            rhs=aT[:, 2 * k2:2 * k2 + 2].rearrange(
                                "p d b q -> p d (b q)"),
                            start=(k2 == 0), stop=(k2 == DT // 2 - 1),
                            perf_mode=DR)
                    t = sq_pool.tile([128, BQ], F32, tag="lns", name="prt")
                    nc.vector.tensor_scalar(
                        t, ps[:, :BQ], 1.0 / (AV_SCALE * FP8_SCALE),
                        w["bp"][:, nt:nt + 1], op0=ALU.mult, op1=ALU.add)
                    nc.vector.tensor_tensor(
                        xT[:, nt].rearrange("p b q -> p (b q)"),
                        xT[:, nt].rearrange("p b q -> p (b q)"), t, ALU.add)

                # ---- LN1 stats, then next layer's kT (fills the gap) ----
                mu_bc, rstd_bc = ln_stats()
                wn = None
                if l + 1 < L:
                    wn = emit_layer_weights(l + 1)
                    kTb_n = emit_kT(wn, wn["know0"])
                g1s, b1s, g2s, b2s, bfs = (w["g1"], w["b1"], w["g2"],
                                           w["b2"], w["bf"])
                ln_apply(mu_bc, rstd_bc, g1s, b1s, hbf)

                # ---- ffn in + gelu ----
                for nt in range(FT):
                    wfc = wch_pool.tile([128, DT, 128], BF16, tag="wf")
                    nc.sync.dma_start(out=wfc, in_=Wf[l, nt])
                    ps = psA.tile([128, 512], F32, tag="psA", name="psf")
                    for kt in range(DT):
                        nc.tensor.matmul(
                            ps[:, :BQ], lhsT=wfc[:, kt],
                            rhs=hbf[:, kt].rearrange("p b q -> p (b q)"),
                            start=(kt == 0), stop=(kt == DT - 1))
                    nc.scalar.activation(
                        out=gT[:, nt].rearrange("p b q -> p (b q)"),
                        in_=ps[:, :BQ], func=AF.Gelu_apprx_tanh,
                        bias=bfs[:, nt:nt + 1])

                # ---- ffn out + residual ----
                for nt in range(DT):
                    wmc = wm_pool.tile([128, FT, 128], BF16, tag="wm")
                    nc.sync.dma_start(out=wmc, in_=Wm[l, nt])
                    ps = psA.tile([128, 512], F32, tag="psA", name="psm")
                    for kt in range(FT):
                        nc.tensor.matmul(
                            ps[:, :BQ], lhsT=wmc[:, kt],
                            rhs=gT[:, kt].rearrange("p b q -> p (b q)"),
                            start=(kt == 0), stop=(kt == FT - 1))
                    nc.vector.scalar_tensor_tensor(
                        out=xT[:, nt].rearrange("p b q -> p (b q)"),
                        in0=ps[:, :BQ], scalar=w["bm"][:, nt:nt + 1],
                        in1=xT[:, nt].rearrange("p b q -> p (b q)"),
                        op0=ALU.add, op1=ALU.add)

                # ---- LN2 stats, then next layer's V (fills the gap) ----
                mu_bc, rstd_bc = ln_stats()
                if l + 1 < L:
                    vb_n = emit_V(wn, wn["know0"])
                    kTb, vb, w = kTb_n, vb_n, wn
                ln_apply(mu_bc, rstd_bc, g2s, b2s, hq8)

            # epilogue: residual out (transposed; host un-transposes)
            nc.sync.dma_start(out=out_ext[:, :, :, :], in_=xT)

    return nc


_CACHE = {}


def _prep(inputs):
    """Host-side layout/dtype prep. Returns per-core in_maps."""
    import ml_dtypes

    bf16 = ml_dtypes.bfloat16
    fp8 = ml_dtypes.float8_e4m3
    f32 = np.float32

    x = np.asarray(inputs["input_ids"], f32) + np.asarray(
        inputs["pos_embed"], f32)[None]
    know = np.asarray(inputs["input_ids_know"], f32)
    Wa = np.asarray(inputs["W_attn"], f32)
    ba = np.asarray(inputs["b_attn"], f32)
    Wpm = np.asarray(inputs["W_proj_attn"], f32)
    bpm = np.asarray(inputs["b_proj_attn"], f32)
    Wfm = np.asarray(inputs["W_fc"], f32)
    bfm = np.asarray(inputs["b_fc"], f32)
    Wmm = np.asarray(inputs["W_proj_mlp"], f32)
    bmm = np.asarray(inputs["b_proj_mlp"], f32)

    def pt(a):  # [L, D'] -> [L, 128, T] with element [l, p, t] = a[l, t*128+p]
        return np.ascontiguousarray(
            a.reshape(L, -1, 128).transpose(0, 2, 1), f32)

    shared = {
        # Wk[l, nt, p, dt, n] = Wa[l, dt*128+p, D + nt*128+n]  (x64, fp8)
        "Wk": np.ascontiguousarray(
            (Wa[:L, :, D:2 * D] * 64.0).reshape(L, DT, 128, DT, 128)
            .transpose(0, 3, 2, 1, 4).astype(fp8)),
        # Wv[l, p, dt, n] = Wa[l, dt*128+p, 2D + n]  (x64, fp8)
        "Wv": np.ascontiguousarray(
            (Wa[:L, :, 2 * D:] * 64.0).reshape(L, DT, 128, D)
            .transpose(0, 2, 1, 3).astype(fp8)),
        # Wp[l, nt, p, kt, n] = Wp[l, kt*128+p, nt*128+n]  (x64, fp8)
        "Wp": np.ascontiguousarray(
            (Wpm[:L] * 64.0).reshape(L, DT, 128, DT, 128)
            .transpose(0, 3, 2, 1, 4).astype(fp8)),
        "Wf": np.ascontiguousarray(
            Wfm[:L].reshape(L, DT, 128, FT, 128)
            .transpose(0, 3, 2, 1, 4).astype(bf16)),
        "Wm": np.ascontiguousarray(
            Wmm[:L].reshape(L, FT, 128, DT, 128)
            .transpose(0, 3, 2, 1, 4).astype(bf16)),
        "bk": pt(ba[:L, D:2 * D] * 64.0),
        "bv": np.ascontiguousarray((ba[:L, 2 * D:] * 64.0).astype(bf16)),
        "bp": pt(bpm[:L]),
        "bf": pt(bfm[:L]),
        "bm": pt(bmm[:L]),
        "g1": pt(np.asarray(inputs["ln1_g"], f32)[:L]),
        "b1": pt(np.asarray(inputs["ln1_b"], f32)[:L]),
        "g2": pt(np.asarray(inputs["ln2_g"], f32)[:L]),
        "b2": pt(np.asarray(inputs["ln2_b"], f32)[:L]),
    }

    in_maps = []
    for c in range(N_CORES):
        m = dict(shared)
        xs = x[c * BL:(c + 1) * BL]  # [BL, SQ, D]
        # xT[p, dt, b, q] = xs[b, q, dt*128+p]
        m["xT0"] = np.ascontiguousarray(
            xs.reshape(BL, SQ, DT, 128).transpose(3, 2, 0, 1), f32)
        ks = know[c * BL:(c + 1) * BL]  # [BL, SKV, D]
        # knowT[b, p, dt, s] = ks[b, s, dt*128+p]
        m["knowT"] = np.ascontiguousarray(
            (ks * 64.0).reshape(BL, SKV, DT, 128)
            .transpose(0, 3, 2, 1).astype(fp8))
        in_maps.append(m)
    return in_maps


def kernel(**inputs):
    if "nc" not in _CACHE:
        _CACHE["nc"] = build_nc()
    nc = _CACHE["nc"]

    in_maps = _prep(inputs)
    _CACHE["last_in_maps"] = in_maps

    res = run_bass_kernel_spmd(nc, in_maps, list(range(N_CORES)))
    outs = []
    for c in range(N_CORES):
        oT = np.asarray(res.results[c]["out"])  # [128, DT, BL, SQ]
        # out[b, q, dt*128+p] = oT[p, dt, b, q]
        outs.append(oT.transpose(2, 3, 1, 0).reshape(BL, SQ, D))
    return np.ascontiguousarray(np.concatenate(outs, axis=0), np.float32)


# revision 22
# speedup vs baseline: 1.8053x; 1.0131x over previous
"""Trainium2 Bass kernel for nn_ReasonerModel (12-layer cross-attn transformer).

Sharding: data-parallel over batch. 32 batch elems / 8 cores = 4 per core.
Each core streams the full weights (host-precast bf16, pre-tiled layouts)
and computes its 4 batch rows end-to-end; no collectives.

v2 design: everything lives in TRANSPOSED space (features on partitions,
tokens on the free axis) - zero PE transposes.
  xT      [128, 8, 4, 80] f32   residual stream (d on partitions)
  hbf     [128, 8, 4, 80] bf16  bf16 cast feeding matmuls (q, then p)
  know_b  [128, 8, 1024] bf16   d-on-partitions know, streamed per (l,b)
  kT_b    [128, 8, 1024] bf16   K^T per b (n on partitions, s free)
  vb      [128, 8, 1024] bf16   V per b (s on partitions, n free)
  wT_b    [128, 8, 16, 80] bf16 exp(scores^T) (s on partitions)
  aT      [128, 8, 4, 80] bf16  attention out (n on partitions)
  gT      [128, 32, 4, 80] bf16 gelu(fc) (4D-features on partitions)
Attention computes scores TRANSPOSED directly (lhsT = k-chunk, rhs = q),
softmax denominators via ones-vector matmuls, and folds 1/sum into the
AV psum drain using DMA-broadcast reciprocals (DRAM bounce).
LayerNorm stats (sum x, sum x^2) via ones-vector matmuls over partitions;
mu/rstd broadcast back via DRAM bounce; apply fully in transposed space.
"""

import os
import sys

sys.path.insert(0, "/opt/trn_rl_repo")

import numpy as np

import concourse.bass as bass
import concourse.tile as tile
from concourse import mybir
from concourse.bass_utils import run_bass_kernel_spmd
from concourse.vector_clock import ScopedClock

# model dims (fixed by the problem)
B, SQ, SKV, D, H = 32, 80, 1024, 1024, 16
L = int(os.environ.get("KERNEL_LAYERS", "12"))
HD = D // H          # 64
N_CORES = 8
BL = B // N_CORES    # 4 batch rows per core
DT = D // 128        # 8 d-tiles
FT = 4 * D // 128    # 32 ffn tiles
BQ = BL * SQ         # 320
EPS = 1e-5
SCALE = 1.0 / np.sqrt(HD)

F32 = mybir.dt.float32
BF16 = mybir.dt.bfloat16
FP8 = mybir.dt.float8e4
AF = mybir.ActivationFunctionType
ALU = mybir.AluOpType
FP8_SCALE = 64.0           # host prescales know + all weights into e4m3 range
FP8_INV = 1.0 / (FP8_SCALE * FP8_SCALE)
AV_SCALE = 4096.0          # aT carries 4096*a so fp8 stays in normal range
LN4096 = float(np.log(AV_SCALE / FP8_SCALE))
DR = mybir.MatmulPerfMode.DoubleRow


class PatchedTC(tile.TileContext):
    """This container's walrus accepts at most ONE sem wait per instruction;
    Tile may attach several. Peel extras onto preceding same-engine no-ops."""

    def _commit_instruction(self, inst, lazy_reg_writes: bool = True):
        si = getattr(inst, "sync_info", None)
        if (
            si is not None
            and si.on_wait
            and len(si.on_wait) > 1
            and inst.engine != mybir.EngineType.Unassigned
        ):
            waits = list(si.on_wait)
            si.on_wait = [waits[-1]]
            for j, w in enumerate(waits[:-1]):
                nop = mybir.InstNoOp(
                    name=f"{inst.name}-sw{j}",
                    sync_info=mybir.SyncInfo(on_wait=[w], on_update=[]),
                    bass_nofuse=True,
                    engine=inst.engine,
                )
                super()._commit_instruction(nop, lazy_reg_writes=False)
        return super()._commit_instruction(inst, lazy_reg_writes)

    def _drain_and_barrier(self, tick_clock, wait_clock):
        drain_inst = self.nc.sync.drain()
        wait_clock.add_sem_waits(
            drain_inst.ins, ScopedClock({None: tick_clock.global_clock})
        )
        si = drain_inst.ins.sync_info
        if si is not None and si.on_wait and len(si.on_wait) > 1:
            waits = list(si.on_wait)
            si.on_wait = waits[:1]
            for w in waits[1:]:
                extra = self.nc.sync.drain()
                nsi = extra.ins.sync_info
                if nsi is None:
                    extra.ins.sync_info = mybir.SyncInfo(on_wait=[w], on_update=[])
                else:
                    nsi.on_wait = [w]
        self.nc.all_engine_barrier()
        assert self.sems is not None
        popped = self.nc._tile_sem_poison_stack.pop()
        assert popped is self._sem_poison
        self.nc.clear_and_free_semaphores(list(self.sems.allocated().values()))
        self.nc.all_engine_barrier()


def bcast_ap(ap_1d, p):
    """Partition-broadcast a 1-D DRAM AP to [p, n] (stride-0 partition dim)."""
    return bass.AP(
        tensor=ap_1d.tensor, offset=ap_1d.offset, ap=[[0, p]] + list(ap_1d.ap)
    )


def build_nc():
    try:  # lift the stale 192KB/partition SBUF cap to the real usable 208KB
        from concourse import tile_utils

        tile_utils.max_sbuf_usage = 208 * 1024
    except Exception:
        pass

    nc = bass.Bass("TRN2", target_bir_lowering=False, debug=False,
                   num_devices=N_CORES)

    # ---- DRAM I/O (host-prepped layouts; see _prep() below) ----
    xT_in = nc.dram_tensor("xT0", [128, DT, BL, SQ], F32, kind="ExternalInput")
    knowT = nc.dram_tensor("knowT", [BL, 128, DT, SKV], FP8,
                           kind="ExternalInput")
    Wk = nc.dram_tensor("Wk", [L, DT, 128, DT, 128], FP8, kind="ExternalInput")
    Wv = nc.dram_tensor("Wv", [L, 128, DT, D], FP8, kind="ExternalInput")
    Wp = nc.dram_tensor("Wp", [L, DT, 128, DT, 128], FP8, kind="ExternalInput")
    Wf = nc.dram_tensor("Wf", [L, FT, 128, DT, 128], BF16, kind="ExternalInput")
    Wm = nc.dram_tensor("Wm", [L, DT, 128, FT, 128], BF16, kind="ExternalInput")
    bk = nc.dram_tensor("bk", [L, 128, DT], F32, kind="ExternalInput")
    bv = nc.dram_tensor("bv", [L, D], BF16, kind="ExternalInput")
    bp = nc.dram_tensor("bp", [L, 128, DT], F32, kind="ExternalInput")
    bf = nc.dram_tensor("bf", [L, 128, FT], F32, kind="ExternalInput")
    bm = nc.dram_tensor("bm", [L, 128, DT], F32, kind="ExternalInput")
    g1 = nc.dram_tensor("g1", [L, 128, DT], F32, kind="ExternalInput")
    b1 = nc.dram_tensor("b1", [L, 128, DT], F32, kind="ExternalInput")
    g2 = nc.dram_tensor("g2", [L, 128, DT], F32, kind="ExternalInput")
    b2 = nc.dram_tensor("b2", [L, 128, DT], F32, kind="ExternalInput")
    out_ext = nc.dram_tensor("out", [128, DT, BL, SQ], F32,
                             kind="ExternalOutput")

    with PatchedTC(nc) as tc:
        import contextlib

        ctx = contextlib.ExitStack()
        with ctx:
            P = lambda **kw: ctx.enter_context(tc.tile_pool(**kw))
            singles = P(name="singles", bufs=1)
            kv_pool = P(name="kv", bufs=2)       # kT_b + vb
            wT_pool = P(name="wT", bufs=1)
            wkv_pool = P(name="wkv", bufs=1)
            wch_pool = P(name="wch", bufs=2)     # wp/wf chunks
            wm_pool = P(name="wm", bufs=2)       # wm chunks (bigger)
            bc_pool = P(name="bc", bufs=2)       # broadcast tiles
            sb_pool = P(name="sb", bufs=2)       # per-layer small biases
            stA_pool = P(name="stA", bufs=1)     # LN tiny stats
            stB_pool = P(name="stB", bufs=2)     # softmax recip tiles
            sq_pool = P(name="sq", bufs=2)       # x^2 / LN scratch
            psA = P(name="psA", bufs=3, space="PSUM")  # [128,512] kv/proj/fc/mlp
            psS = P(name="psS", bufs=2, space="PSUM")  # [128,4,80] scoresT
            psV = P(name="psV", bufs=2, space="PSUM")  # [128,160] AV
            psM = P(name="psM", bufs=1, space="PSUM")  # [1,*] sums/LN stats

            # ---- constants ----
            ones_bf = singles.tile([128, 1], BF16)
            nc.vector.memset(ones_bf, 1.0)
            ones_f8 = singles.tile([128, 1], FP8)
            nc.vector.memset(ones_f8, 1.0)
            ones_f32 = singles.tile([128, 1], F32)
            nc.vector.memset(ones_f32, 1.0)
            ones_row = singles.tile([1, 128], F32)
            nc.vector.memset(ones_row, 1.0)
            eps_t = singles.tile([1, 1], F32)
            nc.vector.memset(eps_t, EPS)
            ln4096_t = singles.tile([1, 1], F32)
            nc.vector.memset(ln4096_t, LN4096)

            # ---- persistent activations ----
            xT = singles.tile([128, DT, BL, SQ], F32, tag="xT")
            nc.sync.dma_start(out=xT, in_=xT_in[:, :, :, :])
            know_res = singles.tile([128, BL, DT, SKV], FP8, tag="know")
            for kb in range(BL):
                nc.sync.dma_start(out=know_res[:, kb], in_=knowT[kb])
            # hbf holds the bf16 cast of the residual: q before attention,
            # then p (LN1 out) for the MLP, then LN2 out = next layer's q.
            hbf = singles.tile([128, DT, BL, SQ], BF16, tag="hbf")
            hq8 = singles.tile([128, DT, BL, SQ], FP8, tag="hq8")
            for dt in range(DT):
                nc.vector.tensor_copy(out=hq8[:, dt], in_=xT[:, dt])

            aT = singles.tile([128, DT, BL, SQ], FP8, tag="aT")
            gT = singles.tile([128, FT, BL, SQ], BF16, tag="gT")

            def ln_stats():
                """LN stats over the partition(d) axis of xT; returns psum
                broadcast tiles (mu_bc, rstd_bc) [128, BQ]."""
                ps_s = psM.tile([1, BQ], F32, tag="psM", name="ps_s")
                ps_q = psS.tile([1, BQ], F32, tag="psS", name="ps_q")
                for dt in range(DT):
                    x2 = xT[:, dt].rearrange("p b q -> p (b q)")
                    xsq = sq_pool.tile([128, BQ], F32, tag="lns", name="xsq")
                    nc.scalar.activation(out=xsq, in_=x2, func=AF.Square)
                    nc.tensor.matmul(
                        ps_s, lhsT=ones_f32, rhs=x2,
                        start=(dt == 0), stop=(dt == DT - 1))
                    nc.tensor.matmul(
                        ps_q, lhsT=ones_f32, rhs=xsq,
                        start=(dt == 0), stop=(dt == DT - 1))
                # mu = ps_s/D ; var = ps_q/D - mu^2 ; rstd = 1/sqrt(var+eps)
                mu = stA_pool.tile([1, BQ], F32, tag="mu")
                nc.vector.tensor_scalar_mul(mu, ps_s, 1.0 / D)
                musq = stA_pool.tile([1, BQ], F32, tag="musq")
                nc.vector.tensor_tensor(musq, mu, mu, ALU.mult)
                var = stA_pool.tile([1, BQ], F32, tag="var")
                nc.vector.scalar_tensor_tensor(
                    out=var, in0=ps_q, scalar=1.0 / D, in1=musq,
                    op0=ALU.mult, op1=ALU.subtract)
                # rstd = exp(-0.5*ln(var+eps))  (Reciprocal/Rsqrt LUTs are
                # unavailable in this container's walrus)
                lnv = stA_pool.tile([1, BQ], F32, tag="lnv")
                nc.scalar.activation(lnv, var, AF.Ln, bias=eps_t)
                rstd = stA_pool.tile([1, BQ], F32, tag="rstd")
                nc.scalar.activation(rstd, lnv, AF.Exp, scale=-0.5)
                # PE-broadcast to all partitions (psum-resident)
                mu_bc = psV.tile([128, BQ], F32, tag="psV", name="mu_bc")
                nc.tensor.matmul(mu_bc, lhsT=ones_row, rhs=mu,
                                 start=True, stop=True)
                rstd_bc = psV.tile([128, BQ], F32, tag="psV", name="rstd_bc")
                nc.tensor.matmul(rstd_bc, lhsT=ones_row, rhs=rstd,
                                 start=True, stop=True)
                return mu_bc, rstd_bc

            def ln_apply(mu_bc, rstd_bc, g_sb, b_sb, cast_out):
                """x = (x - mu)*rstd*g + b ; cast_out = lowprec(x)."""
                for dt in range(DT):
                    x2 = xT[:, dt].rearrange("p b q -> p (b q)")
                    t = sq_pool.tile([128, BQ], F32, tag="lns", name="lnt")
                    nc.vector.tensor_tensor(t, x2, mu_bc, ALU.subtract)
                    nc.vector.tensor_tensor(t, t, rstd_bc, ALU.mult)
                    nc.vector.tensor_scalar(
                        x2, t, g_sb[:, dt:dt + 1], b_sb[:, dt:dt + 1],
                        op0=ALU.mult, op1=ALU.add)
                    h2 = cast_out[:, dt].rearrange("p b q -> p (b q)")
                    if dt % 2 == 0:
                        nc.scalar.copy(out=h2, in_=x2)
                    else:
                        nc.vector.tensor_copy(out=h2, in_=x2)

            def emit_layer_weights(l):
                """DMA layer-l kv weights + biases; returns handle dict."""
                w = {}
                w["wk"] = wkv_pool.tile([128, DT, DT, 128], FP8, tag="wk", name="wk")
                nc.sync.dma_start(
                    out=w["wk"], in_=Wk[l].rearrange("t p d n -> p t d n"))
                w["wv"] = wkv_pool.tile([128, DT, D], FP8, tag="wv", name="wv")
                nc.sync.dma_start(out=w["wv"], in_=Wv[l])
                for nm, src in [("bk", bk), ("bp", bp), ("bm", bm),
                                ("g1", g1), ("b1", b1), ("g2", g2),
                                ("b2", b2)]:
                    w[nm] = sb_pool.tile([128, DT], F32, tag=nm, name=nm)
                    nc.sync.dma_start(out=w[nm], in_=src[l])
                w["bf"] = sb_pool.tile([128, FT], F32, tag="bf", name="bfs")
                nc.sync.dma_start(out=w["bf"], in_=bf[l])
                w["bv"] = bc_pool.tile([128, D], BF16, tag="bv", name="bv")
                nc.gpsimd.dma_start(out=w["bv"], in_=bcast_ap(bv[l], 128))
                return w

            def emit_kT(w, b):
                """K^T [n-part, s] for one b."""
                kTb = kv_pool.tile([128, DT, SKV], FP8, tag="kT")
                for nt in range(DT):
                    for sc in range(2):
                        ps = psA.tile([128, 512], F32, tag="psA", name="psk")
                        for k2 in range(DT // 2):
                            nc.tensor.matmul(
                                ps, lhsT=w["wk"][:, nt, 2 * k2:2 * k2 + 2, :],
                                rhs=know_res[:, b, 2 * k2:2 * k2 + 2,
                                             sc * 512:(sc + 1) * 512],
                                start=(k2 == 0), stop=(k2 == DT // 2 - 1),
                                perf_mode=DR)
                        if sc == 0:
                            nc.scalar.activation(
                                out=kTb[:, nt, sc * 512:(sc + 1) * 512],
                                in_=ps, func=AF.Identity,
                                scale=1.0 / FP8_SCALE,
                                bias=w["bk"][:, nt:nt + 1])
                        else:
                            nc.vector.tensor_scalar(
                                kTb[:, nt, sc * 512:(sc + 1) * 512], ps,
                                1.0 / FP8_SCALE, w["bk"][:, nt:nt + 1],
                                op0=ALU.mult, op1=ALU.add)
                return kTb

            def emit_V(w, b):
                """V [s-part, n] for one b."""
                vb = kv_pool.tile([128, DT, D], FP8, tag="v")
                for sv in range(DT):
                    for nh in range(2):
                        ps = psA.tile([128, 512], F32, tag="psA", name="psv")
                        for k2 in range(DT // 2):
                            nc.tensor.matmul(
                                ps,
                                lhsT=know_res[:, b, 2 * k2:2 * k2 + 2,
                                              sv * 128:(sv + 1) * 128],
                                rhs=w["wv"][:, 2 * k2:2 * k2 + 2,
                                            nh * 512:(nh + 1) * 512],
                                start=(k2 == 0), stop=(k2 == DT // 2 - 1),
                                perf_mode=DR)
                        nc.vector.scalar_tensor_tensor(
                            out=vb[:, sv, nh * 512:(nh + 1) * 512],
                            in0=ps, scalar=1.0 / FP8_SCALE,
                            in1=w["bv"][:, nh * 512:(nh + 1) * 512],
                            op0=ALU.mult, op1=ALU.add)
                return vb

            def emit_scores(b, kTb):
                """scores^T -> exp into wT_b [s-part, sc, h, q]."""
                wTb = wT_pool.tile([128, DT, H, SQ], FP8, tag="wT")
                for h in range(H):
                    po = (h % 2) * 64
                    hp = h // 2
                    for g in range(2):
                        ps = psS.tile([128, 4, SQ], F32, tag="psS", name="pss")
                        for j in range(4):
                            sc = g * 4 + j
                            nc.tensor.matmul(
                                ps[:, j, :],
                                lhsT=kTb[po:po + 64, hp,
                                         sc * 128:(sc + 1) * 128],
                                rhs=hq8[po:po + 64, hp, b, :],
                                start=True, stop=True)
                        nc.scalar.activation(
                            out=wTb[:, g * 4:(g + 1) * 4, h, :],
                            in_=ps, func=AF.Exp, scale=SCALE / FP8_SCALE)
                return wTb

            def emit_sums(b, wTb):
                """softmax sums -> (AV_SCALE/64)/sum -> PE-bcast -> sbuf."""
                rs_sb = bc_pool.tile([128, H, SQ], BF16, tag="rssb")
                for hg in range(4):
                    if hg % 2 == 0:
                        ps = psM.tile([1, 4 * SQ], F32, tag="psM",
                                      name="pssum")
                    else:
                        ps = psS.tile([1, 4 * SQ], F32, tag="psS",
                                      name="pssum2")
                    for sc in range(DT):
                        nc.tensor.matmul(
                            ps, lhsT=ones_f8,
                            rhs=wTb[:, sc, hg * 4:(hg + 1) * 4, :]
                            .rearrange("p h q -> p (h q)"),
                            start=(sc == 0), stop=(sc == DT - 1))
                    lnp = stB_pool.tile([1, 4 * SQ], F32, tag="lnp")
                    nc.scalar.activation(lnp, ps, AF.Ln)
                    rs = stB_pool.tile([1, 4 * SQ], F32, tag="rs")
                    nc.scalar.activation(rs, lnp, AF.Exp, scale=-1.0,
                                         bias=ln4096_t)
                    bcp = psS.tile([128, 4 * SQ], F32, tag="psS", name="bcp")
                    nc.tensor.matmul(bcp, lhsT=ones_row, rhs=rs,
                                     start=True, stop=True)
                    nc.scalar.copy(
                        out=rs_sb[:, hg * 4:(hg + 1) * 4, :]
                        .rearrange("p h q -> p (h q)"), in_=bcp)
                return rs_sb

            def emit_AV(b, wTb, vb, rs_sb):
                """AV (head pairs) + normalize into aT[:, :, b, :]."""
                for hp in range(DT):
                    ps = psV.tile([128, 2 * SQ], F32, tag="psV", name="psav")
                    for sv in range(DT):
                        nc.tensor.matmul(
                            ps,
                            lhsT=vb[:, sv, hp * 128:(hp + 1) * 128],
                            rhs=wTb[:, sv, 2 * hp:2 * hp + 2, :].rearrange(
                                "p h q -> p (h q)"),
                            start=(sv == 0), stop=(sv == DT - 1))
                    nc.vector.tensor_tensor(
                        aT[0:64, hp, b, :], ps[0:64, 0:SQ],
                        rs_sb[0:64, 2 * hp, :], ALU.mult)
                    nc.vector.tensor_tensor(
                        aT[64:128, hp, b, :], ps[64:128, SQ:2 * SQ],
                        rs_sb[64:128, 2 * hp + 1, :], ALU.mult)

            # ================= layers (kv software-pipelined) =================
            w = emit_layer_weights(0)
            kTb = emit_kT(w, 0)
            vb = emit_V(w, 0)
            for l in range(L):
                for b in range(BL):
                    wTb = emit_scores(b, kTb)
                    if b + 1 < BL:
                        kTb2 = emit_kT(w, b + 1)
                    rs_sb = emit_sums(b, wTb)
                    if b + 1 < BL:
                        vb2 = emit_V(w, b + 1)
                    emit_AV(b, wTb, vb, rs_sb)
                    if b + 1 < BL:
                        kTb, vb = kTb2, vb2

                # ---- attention out-projection + residual ----
                for nt in range(DT):
                    wpc = wch_pool.tile([128, DT, 128], FP8, tag="wp")
                    nc.sync.dma_start(out=wpc, in_=Wp[l, nt])
                    ps = psA.tile([128, 512], F32, tag="psA", name="psp")
                    for k2 in range(DT // 2):
                        nc.tensor.matmul(
                            ps[:, :BQ], lhsT=wpc[:, 2 * k2:2 * k2 + 2, :],
                            rhs=aT[:, 2 * k2:2 * k2 + 2].rearrange(
                                "p d b q -> p d (b q)"),
                            start=(k2 == 0), stop=(k2 == DT // 2 - 1),
                            perf_mode=DR)
                    t = sq_pool.tile([128, BQ], F32, tag="lns", name="prt")
                    nc.vector.tensor_scalar(
                        t, ps[:, :BQ], 1.0 / (AV_SCALE * FP8_SCALE),
                        w["bp"][:, nt:nt + 1], op0=ALU.mult, op1=ALU.add)
                    nc.vector.tensor_tensor(
                        xT[:, nt].rearrange("p b q -> p (b q)"),
                        xT[:, nt].rearrange("p b q -> p (b q)"), t, ALU.add)

                # ---- LN1 stats, then next layer's kT (fills the gap) ----
                mu_bc, rstd_bc = ln_stats()
                wn = None
                if l + 1 < L:
                    wn = emit_layer_weights(l + 1)
                    kTb_n = emit_kT(wn, 0)
                g1s, b1s, g2s, b2s, bfs = (w["g1"], w["b1"], w["g2"],
                                           w["b2"], w["bf"])
                ln_apply(mu_bc, rstd_bc, g1s, b1s, hbf)

                # ---- ffn in + gelu ----
                for nt in range(FT):
                    wfc = wch_pool.tile([128, DT, 128], BF16, tag="wf")
                    nc.sync.dma_start(out=wfc, in_=Wf[l, nt])
                    ps = psA.tile([128, 512], F32, tag="psA", name="psf")
                    for kt in range(DT):
                        nc.tensor.matmul(
                            ps[:, :BQ], lhsT=wfc[:, kt],
                            rhs=hbf[:, kt].rearrange("p b q -> p (b q)"),
                            start=(kt == 0), stop=(kt == DT - 1))
                    nc.scalar.activation(
                        out=gT[:, nt].rearrange("p b q -> p (b q)"),
                        in_=ps[:, :BQ], func=AF.Gelu_apprx_tanh,
                        bias=bfs[:, nt:nt + 1])

                # ---- ffn out + residual ----
                for nt in range(DT):
                    wmc = wm_pool.tile([128, FT, 128], BF16, tag="wm")
                    nc.sync.dma_start(out=wmc, in_=Wm[l, nt])
                    ps = psA.tile([128, 512], F32, tag="psA", name="psm")
                    for kt in range(FT):
                        nc.tensor.matmul(
                            ps[:, :BQ], lhsT=wmc[:, kt],
                            rhs=gT[:, kt].rearrange("p b q -> p (b q)"),
                            start=(kt == 0), stop=(kt == FT - 1))
                    nc.vector.scalar_tensor_tensor(
                        out=xT[:, nt].rearrange("p b q -> p (b q)"),
                        in0=ps[:, :BQ], scalar=w["bm"][:, nt:nt + 1],
                        in1=xT[:, nt].rearrange("p b q -> p (b q)"),
                        op0=ALU.add, op1=ALU.add)

                # ---- LN2 stats, then next layer's V (fills the gap) ----
                mu_bc, rstd_bc = ln_stats()
                if l + 1 < L:
                    vb_n = emit_V(wn, 0)
                    kTb, vb, w = kTb_n, vb_n, wn
                ln_apply(mu_bc, rstd_bc, g2s, b2s, hq8)

            # epilogue: residual out (transposed; host un-transposes)
            nc.sync.dma_start(out=out_ext[:, :, :, :], in_=xT)

    return nc


_CACHE = {}


def _prep(inputs):
    """Host-side layout/dtype prep. Returns per-core in_maps."""
    import ml_dtypes

    bf16 = ml_dtypes.bfloat16
    fp8 = ml_dtypes.float8_e4m3
    f32 = np.float32

    x = np.asarray(inputs["input_ids"], f32) + np.asarray(
        inputs["pos_embed"], f32)[None]
    know = np.asarray(inputs["input_ids_know"], f32)
    Wa = np.asarray(inputs["W_attn"], f32)
    ba = np.asarray(inputs["b_attn"], f32)
    Wpm = np.asarray(inputs["W_proj_attn"], f32)
    bpm = np.asarray(inputs["b_proj_attn"], f32)
    Wfm = np.asarray(inputs["W_fc"], f32)
    bfm = np.asarray(inputs["b_fc"], f32)
    Wmm = np.asarray(inputs["W_proj_mlp"], f32)
    bmm = np.asarray(inputs["b_proj_mlp"], f32)

    def pt(a):  # [L, D'] -> [L, 128, T] with element [l, p, t] = a[l, t*128+p]
        return np.ascontiguousarray(
            a.reshape(L, -1, 128).transpose(0, 2, 1), f32)

    shared = {
        # Wk[l, nt, p, dt, n] = Wa[l, dt*128+p, D + nt*128+n]  (x64, fp8)
        "Wk": np.ascontiguousarray(
            (Wa[:L, :, D:2 * D] * 64.0).reshape(L, DT, 128, DT, 128)
            .transpose(0, 3, 2, 1, 4).astype(fp8)),
        # Wv[l, p, dt, n] = Wa[l, dt*128+p, 2D + n]  (x64, fp8)
        "Wv": np.ascontiguousarray(
            (Wa[:L, :, 2 * D:] * 64.0).reshape(L, DT, 128, D)
            .transpose(0, 2, 1, 3).astype(fp8)),
        # Wp[l, nt, p, kt, n] = Wp[l, kt*128+p, nt*128+n]  (x64, fp8)
        "Wp": np.ascontiguousarray(
            (Wpm[:L] * 64.0).reshape(L, DT, 128, DT, 128)
            .transpose(0, 3, 2, 1, 4).astype(fp8)),
        "Wf": np.ascontiguousarray(
            Wfm[:L].reshape(L, DT, 128, FT, 128)
            .transpose(0, 3, 2, 1, 4).astype(bf16)),
        "Wm": np.ascontiguousarray(
            Wmm[:L].reshape(L, FT, 128, DT, 128)
            .transpose(0, 3, 2, 1, 4).astype(bf16)),
        "bk": pt(ba[:L, D:2 * D] * 64.0),
        "bv": np.ascontiguousarray((ba[:L, 2 * D:] * 64.0).astype(bf16)),
        "bp": pt(bpm[:L]),
        "bf": pt(bfm[:L]),
        "bm": pt(bmm[:L]),
        "g1": pt(np.asarray(inputs["ln1_g"], f32)[:L]),
        "b1": pt(np.asarray(inputs["ln1_b"], f32)[:L]),
        "g2": pt(np.asarray(inputs["ln2_g"], f32)[:L]),
        "b2": pt(np.asarray(inputs["ln2_b"], f32)[:L]),
    }

    in_maps = []
    for c in range(N_CORES):
        m = dict(shared)
        xs = x[c * BL:(c + 1) * BL]  # [BL, SQ, D]
        # xT[p, dt, b, q] = xs[b, q, dt*128+p]
        m["xT0"] = np.ascontiguousarray(
            xs.reshape(BL, SQ, DT, 128).transpose(3, 2, 0, 1), f32)
        ks = know[c * BL:(c + 1) * BL]  # [BL, SKV, D]
        # knowT[b, p, dt, s] = ks[b, s, dt*128+p]
        m["knowT"] = np.ascontiguousarray(
            (ks * 64.0).reshape(BL, SKV, DT, 128)
            .transpose(0, 3, 2, 1).astype(fp8))
        in_maps.append(m)
    return in_maps


def kernel(**inputs):
    if "nc" not in _CACHE:
        _CACHE["nc"] = build_nc()
    nc = _CACHE["nc"]

    in_maps = _prep(inputs)
    _CACHE["last_in_maps"] = in_maps

    res = run_bass_kernel_spmd(nc, in_maps, list(range(N_CORES)))
    outs = []
    for c in range(N_CORES):
        oT = np.asarray(res.results[c]["out"])  # [128, DT, BL, SQ]
        # out[b, q, dt*128+p] = oT[p, dt, b, q]
        outs.append(oT.transpose(2, 3, 1, 0).reshape(BL, SQ, D))
    return np.ascontiguousarray(np.concatenate(outs, axis=0), np.float32)
